# revision 1
# baseline (speedup 1.0000x reference)
"""Trainium2 Bass kernel for nn_ClassifierGuided (2-modality top-12-of-16 MoE classifier).

Sharding: pure data-parallel over tokens. 2 modalities x 4096 tokens = 8192
tokens; each of the 8 cores owns 1024 tokens of one modality (cores 0-3 ->
modality 0, cores 4-7 -> modality 1) and that modality's full weights.
Dense-eval MoE (all 16 experts computed, sparse gates applied), so no
all-to-all is needed.

Per-core math (transposed activation layout, d on partitions):
  gating   : logits = x @ Wg -> top-12 mask -> softmax -> gates g [B,16]
  experts  : h_e = relu(x @ W1_e + b1_e); hg_e = g_e * h_e
  combine  : moeT = sum_e W2_e^T @ hg_e  (+ b2^T @ g^T folded as one matmul)
  residual : z = relu(moe) + x
  head     : outT = Wo^T @ zT + bo

All matmuls run as float32r (full-rate fp32 PE path, ~1e-4 relative rounding).
Experts are processed in pairs so every expert matmul is a full 128x128 tile
(192+192 = 384 = 3*128 h-dims per pair).
"""
import sys

sys.path.insert(0, "/opt/trn_rl_repo")

import numpy as np

import concourse.bass as bass
import concourse.mybir as mybir
import concourse.tile as tile
from concourse import bacc
from concourse.bass_utils import run_bass_kernel_spmd
from concourse.masks import make_identity

# ---- problem sizes (hardcoded per the harness contract) ----
B = 4096           # tokens per modality
D = 768            # model dim
E = 16             # experts
H = 192            # expert hidden
O = 101            # classifier out
KTOP = 12          # top-k experts
NCORES = 8
BC = B // 4        # 1024 tokens per core
DC = D // 128      # 6 d-chunks
NT = 512           # token tile (matmul moving dim)
NTILES = BC // NT  # 2
NPAIR = E // 2     # 8 expert pairs
HP = 2 * H         # 384 h-dims per pair = 3 chunks of 128
HCH = HP // 128    # 3
F32 = mybir.dt.float32
F32R = mybir.dt.float32r
NEG_BIG = -1.0e30

_NC_CACHE = {}
DEBUG = False


def build_nc():
    nc = bacc.Bacc("TRN2", target_bir_lowering=False, debug=False,
                   num_devices=NCORES)

    # ---- DRAM I/O (per-core views; host pre-packs) ----
    xT = nc.dram_tensor("xT", [D, BC], F32R, kind="ExternalInput").ap()
    w1p = nc.dram_tensor("w1p", [D, E * H], F32R, kind="ExternalInput").ap()
    w2p = nc.dram_tensor("w2p", [E * H, D], F32R, kind="ExternalInput").ap()
    b1p = nc.dram_tensor("b1p", [128, E * H // 128], F32, kind="ExternalInput").ap()
    b2 = nc.dram_tensor("b2", [E, D], F32R, kind="ExternalInput").ap()
    wg = nc.dram_tensor("wg", [D, E], F32R, kind="ExternalInput").ap()
    wo = nc.dram_tensor("wo", [D, O], F32R, kind="ExternalInput").ap()
    bo = nc.dram_tensor("bo", [O, 1], F32, kind="ExternalInput").ap()
    outT = nc.dram_tensor("outT", [O, BC], F32, kind="ExternalOutput").ap()
    if DEBUG:
        dbg_gT = nc.dram_tensor("dbg_gT", [E, BC], F32, kind="ExternalOutput").ap()
        dbg_gb = nc.dram_tensor("dbg_gb", [128, 2, NT], F32, kind="ExternalOutput").ap()
        dbg_hg = nc.dram_tensor("dbg_hg", [128, NT], F32, kind="ExternalOutput").ap()
        dbg_h = nc.dram_tensor("dbg_h", [128, NT], F32, kind="ExternalOutput").ap()
        dbg_moe = nc.dram_tensor("dbg_moe", [128, DC, NT], F32, kind="ExternalOutput").ap()

    with tile.TileContext(nc) as tc:
        with tc.tile_pool(name="const", bufs=1) as cpool:
            # resident SBUF tensors
            xsb = cpool.tile([128, DC, BC], F32R)          # xT; later holds zT
            w1sb = cpool.tile([128, DC, E * H], F32R)
            b1sb = cpool.tile([128, E * H // 128], F32)
            b2sb = cpool.tile([E, D], F32R)
            wosb = cpool.tile([128, DC, O], F32R)
            bosb = cpool.tile([O, 1], F32)
            gT = cpool.tile([E, BC], F32R)                 # gates, expert-major
            wgf = cpool.tile([128, DC, E], F32)            # fp32 gating weights
            ident = cpool.tile([128, 128], F32)

            make_identity(nc, ident[:, :])

            # loads ordered by first use: wg + x (gating) split across the SP
            # and ACT HWDGE queues, then W1 by pair interleaved with the small
            # tensors so pair 0 lands as early as possible
            nc.sync.dma_start(out=wgf[:, :, :],
                              in_=wg.bitcast(F32).rearrange("(c p) e -> p c e", p=128))
            for c in range(DC):
                eng = nc.sync if c < 3 else nc.scalar
                eng.dma_start(out=xsb[:, c, :], in_=xT[128 * c:128 * (c + 1), :])
            w1v = w1p.rearrange("(c q) h -> q c h", q=128)

            def load_w1(p):
                nc.sync.dma_start(out=w1sb[:, :, HP * p:HP * (p + 1)],
                                  in_=w1v[:, :, HP * p:HP * (p + 1)])

            w2tiles = {}
            ctx_w2 = tc.tile_pool(name="w2pool", bufs=3)
            w2pool = ctx_w2.__enter__()

            def load_w2(t, p):
                # W2 on the SP queue (not ACT: transfers there block the
                # latency-critical relu chain); one DMA per pair
                w2 = w2pool.tile([128, HCH, D], F32R, tag="w2", name="w2t")
                nc.sync.dma_start(
                    out=w2[:, :, :],
                    in_=w2p[HP * p:HP * (p + 1), :].rearrange(
                        "(m q) d -> q m d", q=128))
                w2tiles[(t, p)] = w2

            load_w1(0)
            nc.sync.dma_start(out=b1sb[:, :], in_=b1p[:, :])
            load_w1(1)
            load_w2(0, 0)
            load_w1(2)
            load_w2(0, 1)
            nc.sync.dma_start(out=b2sb[:, :], in_=b2[:, :])
            load_w1(3)
            load_w2(0, 2)
            for c in range(DC):
                nc.sync.dma_start(out=wosb[:, c, :], in_=wo[128 * c:128 * (c + 1), :])
            nc.sync.dma_start(out=bosb[:, :], in_=bo[:, :])
            for p in range(4, NPAIR):
                load_w1(p)
                load_w2(0, p - 1)
            load_w2(0, NPAIR - 1)

            # gates round-trip through DRAM; gate-broadcast tiles are read
            # back with a partition-step-0 DMA (POOL partition_broadcast is
            # HW-limited to out-base-partition 0)
            gdram = cpool.tile([E, BC], F32R, space="DRAM")
            gdram_ap = gdram
            gb_pre = {}
            gbpool = ctx_gb = tc.tile_pool(name="gbpool", bufs=2)
            gbpool = ctx_gb.__enter__()

            def load_gb(t, p):
                # gb[:,0,:] = gate(e0) broadcast over partitions; [:,1,:] = e1
                gb = gbpool.tile([128, 2, NT], F32R, tag="gb", name="gb")
                gb_src = bass.AP(tensor=gdram.tensor,
                                 offset=2 * p * BC + NT * t,
                                 ap=[[0, 128], [BC, 2], [1, NT]])
                nc.gpsimd.dma_start(out=gb[:, :, :], in_=gb_src)
                return gb

            # ---------------- gating pass (128-token subtiles) ----------------
            with tc.tile_pool(name="gps", bufs=2, space="PSUM") as gps, \
                 tc.tile_pool(name="gtp", bufs=2, space="PSUM") as gtp, \
                 tc.tile_pool(name="gsb", bufs=3) as gsb, \
                 tc.tile_pool(name="xgpool", bufs=2) as xgpool:
                xTv32 = xT.bitcast(F32).rearrange("(c q) b -> q c b", q=128)
                for i in range(BC // 128):
                    if i * 128 % NT == 0 and i > 0:
                        # flush finished token-half of the gates to DRAM early
                        lo = i * 128 - NT
                        nc.gpsimd.dma_start(out=gdram_ap[:, lo:lo + NT],
                                            in_=gT[:, lo:lo + NT])
                        if lo == 0:
                            gb_pre[(0, 0)] = load_gb(0, 0)
                            gb_pre[(0, 1)] = load_gb(0, 1)
                    ts = slice(128 * i, 128 * (i + 1))
                    # fp32-typed copies so the logits matmul runs in exact fp32
                    # (top-12 selection then matches the fp32 reference)
                    xg = xgpool.tile([128, DC, 128], F32, tag="xg", name="xg")
                    nc.gpsimd.dma_start(out=xg[:, :, :], in_=xTv32[:, :, ts])
                    lg_ps = gps.tile([128, E], F32, tag="lg")
                    for c in range(DC):
                        nc.tensor.matmul(lg_ps[:, :], xg[:, c, :], wgf[:, c, :],
                                         start=(c == 0), stop=(c == DC - 1))
                    lg = gsb.tile([128, E], F32, tag="lg_sb")
                    nc.vector.tensor_copy(lg[:, :], lg_ps[:, :])
                    # top-8 values, then values 9..16 after masking them out
                    t8a = gsb.tile([128, 8], F32, tag="t8a")
                    nc.vector.max(t8a[:, :], lg[:, :])
                    l2 = gsb.tile([128, E], F32, tag="l2")
                    nc.vector.match_replace(l2[:, :], t8a[:, :], lg[:, :], NEG_BIG)
                    t8b = gsb.tile([128, 8], F32, tag="t8b")
                    nc.vector.max(t8b[:, :], l2[:, :])
                    # softmax over entries >= 12th-largest (t8b[:,3])
                    e16 = gsb.tile([128, E], F32, tag="e16")
                    nc.scalar.activation(e16[:, :], lg[:, :],
                                         mybir.ActivationFunctionType.Exp)
                    em = gsb.tile([128, E], F32, tag="em")
                    ssum = gsb.tile([128, 1], F32, tag="ssum")
                    nc.vector.scalar_tensor_tensor(
                        out=em[:, :], in0=lg[:, :], scalar=t8b[:, 3:4],
                        in1=e16[:, :], op0=mybir.AluOpType.is_ge,
                        op1=mybir.AluOpType.mult, accum_out=ssum[:, :])
                    rinv = gsb.tile([128, 1], F32, tag="rinv")
                    nc.vector.reciprocal(rinv[:, :], ssum[:, :])
                    g = gsb.tile([128, E], F32, tag="g")
                    nc.vector.tensor_scalar_mul(g[:, :], em[:, :], rinv[:, :])
                    # transpose to expert-major gT[16, tokens]
                    gt_ps = gtp.tile([E, 128], F32, tag="gt")
                    nc.tensor.transpose(gt_ps[:, :], g[:, :], ident[:, :])
                    nc.vector.tensor_copy(gT[:, ts], gt_ps[:, :])
            if DEBUG:
                nc.sync.dma_start(out=dbg_gT[:, :], in_=gT[:, :].bitcast(F32))

            nc.gpsimd.dma_start(out=gdram_ap[:, BC - NT:], in_=gT[:, BC - NT:])

            # ---------------- main loop ----------------
            with tc.tile_pool(name="moeps", bufs=DC, space="PSUM") as moeps, \
                 tc.tile_pool(name="hps", bufs=2, space="PSUM") as hps, \
                 tc.tile_pool(name="gstpool", bufs=2) as gstpool, \
                 tc.tile_pool(name="hgpool", bufs=(20 if DEBUG else 22)) as hgpool, \
                 tc.tile_pool(name="opool", bufs=2) as opool:
                for t in range(NTILES):
                    ts = slice(NT * t, NT * (t + 1))
                    # one PSUM tile per d-chunk: a single big tile would put a
                    # false tile-level WAR between chunk c's drain (DVE read)
                    # and chunk c+1's accumulation (PE write)
                    moe = [moeps.tile([128, NT], F32, tag="moe", name="moe")
                           for _ in range(DC)]
                    w2t = [None] * NPAIR
                    hg = [[None] * HCH for _ in range(NPAIR)]

                    def stage1(p, ts=ts, moe=moe, w2t=w2t, hg=hg, t=t):
                        w2t[p] = w2tiles.pop((t, p), None)
                        if w2t[p] is None:
                            load_w2(t, p)
                            w2t[p] = w2tiles.pop((t, p))
                        gb = gb_pre.pop((t, p), None)
                        if gb is None:
                            gb = load_gb(t, p)
                        if DEBUG and t == 0 and p == 0:
                            nc.sync.dma_start(out=dbg_gb[:, :, :], in_=gb[:, :, :].bitcast(F32))
                        for m in range(HCH):
                            hcol = HP * p + 128 * m
                            hps_t = hps.tile([128, NT], F32, tag="h")
                            for c in range(DC):
                                nc.tensor.matmul(hps_t[:, :],
                                                 w1sb[:, c, hcol:hcol + 128],
                                                 xsb[:, c, ts],
                                                 start=(c == 0), stop=(c == DC - 1))
                            # relu(u + b1) in-place in PSUM, then gate-multiply to SBUF
                            nc.scalar.activation(hps_t[:, :], hps_t[:, :],
                                                 mybir.ActivationFunctionType.Relu,
                                                 bias=b1sb[:, hcol // 128:hcol // 128 + 1])
                            if DEBUG and t == 0 and p == 0 and m == 0:
                                dbg_h_sb = gstpool.tile([128, NT], F32, tag="dbg", name="dbg_h_sb")
                                nc.vector.tensor_copy(dbg_h_sb[:, :], hps_t[:, :])
                                nc.sync.dma_start(out=dbg_h[:, :], in_=dbg_h_sb[:, :])
                            hg[p][m] = hgpool.tile([128, NT], F32R, tag="hg", name="hg")
                            if m == 1:
                                # mixed chunk: parts 0:64 are e0's h[128:192],
                                # parts 64:128 are e1's h[0:64]
                                nc.vector.tensor_tensor(
                                    out=hg[p][m][0:64, :], in0=hps_t[0:64, :],
                                    in1=gb[0:64, 0, :].bitcast(F32),
                                    op=mybir.AluOpType.mult)
                                nc.vector.tensor_tensor(
                                    out=hg[p][m][64:128, :], in0=hps_t[64:128, :],
                                    in1=gb[64:128, 1, :].bitcast(F32),
                                    op=mybir.AluOpType.mult)
                            else:
                                nc.vector.tensor_tensor(
                                    out=hg[p][m][:, :], in0=hps_t[:, :],
                                    in1=gb[:, 0 if m == 0 else 1, :].bitcast(F32),
                                    op=mybir.AluOpType.mult)
                            if DEBUG and t == 0 and p == 0 and m == 0:
                                nc.sync.dma_start(out=dbg_hg[:, :], in_=hg[p][m][:, :].bitcast(F32))

                    def stage2(p, moe=moe, w2t=w2t, hg=hg, ts=ts, close=False):
                        if not close:
                            # m-outer: the first 12 matmuls need only hg m0/m1,
                            # giving hg m2's relu+mult chain extra cover
                            for m in range(HCH):
                                for c in range(DC):
                                    nc.tensor.matmul(moe[c][:, :],
                                                     w2t[p][:, m, 128 * c:128 * (c + 1)],
                                                     hg[p][m][:, :],
                                                     start=(p == 0 and m == 0), stop=False)
                            return
                        for c in range(DC):
                            for m in range(HCH):
                                nc.tensor.matmul(moe[c][:, :],
                                                 w2t[p][:, m, 128 * c:128 * (c + 1)],
                                                 hg[p][m][:, :],
                                                 start=(p == 0 and m == 0), stop=False)
                            if close:
                                # b2 bias term closes this chunk's accumulation
                                nc.tensor.matmul(moe[c][:, :],
                                                 b2sb[:, 128 * c:128 * (c + 1)],
                                                 gT[:, ts], start=False, stop=True)
                                finish_chunk(c)
                                # head matmul trails two chunks behind so its
                                # relu+residual drain is already complete
                                if c >= 2:
                                    head_chunk(c - 2)
                        if close:
                            head_chunk(DC - 2)
                            head_chunk(DC - 1)

                    def finish_chunk(c, moe=moe, ts=ts):
                        # z = relu(moe) + x in one DVE op, overwriting x in place
                        if DEBUG and t == 0:
                            dbg_moe_sb = gstpool.tile([128, NT], F32, tag="dbg", name="dbg_moe_sb")
                            nc.vector.tensor_copy(dbg_moe_sb[:, :], moe[c][:, :])
                            nc.sync.dma_start(out=dbg_moe[:, c, :], in_=dbg_moe_sb[:, :])
                        nc.vector.scalar_tensor_tensor(
                            out=xsb[:, c, ts], in0=moe[c][:, :], scalar=0.0,
                            in1=xsb[:, c, ts].bitcast(F32),
                            op0=mybir.AluOpType.max, op1=mybir.AluOpType.add)

                    out_ps_box = [None]

                    def head_chunk(c, ts=ts):
                        if out_ps_box[0] is None:
                            out_ps_box[0] = hps.tile([O, NT], F32, tag="h",
                                                     name="out_ps")
                        nc.tensor.matmul(out_ps_box[0][:, :], wosb[:, c, :],
                                         xsb[:, c, ts],
                                         start=(c == 0), stop=(c == DC - 1))

                    # software pipeline: stage1(p+1) covers stage2(p) latency;
                    # the last pair closes each moe chunk so relu/residual/head
                    # drain per chunk while later chunks still accumulate
                    stage1(0)
                    for p in range(NPAIR):
                        if p + 1 < NPAIR:
                            stage1(p + 1)
                        stage2(p, close=(p == NPAIR - 1))
                    out_ps = out_ps_box[0]
                    osb = opool.tile([O, NT], F32, tag="osb")
                    nc.scalar.activation(osb[:, :], out_ps[:, :],
                                         mybir.ActivationFunctionType.Identity,
                                         bias=bosb[:, :])
                    nc.sync.dma_start(out=outT[:, ts], in_=osb[:, :])
            ctx_gb.__exit__(None, None, None)
            ctx_w2.__exit__(None, None, None)

    nc.compile()
    return nc


def _pack_core_inputs(x, Wg, W1, b1, W2, b2, Wo, bo, c4):
    """Per-core input dict for one modality's weights + 1024-token slice."""
    f = np.float32
    tok = slice(BC * c4, BC * (c4 + 1))
    return {
        "xT": np.ascontiguousarray(np.asarray(x[tok], f).T),
        "w1p": np.ascontiguousarray(np.asarray(W1, f).transpose(1, 0, 2).reshape(D, E * H)),
        "w2p": np.ascontiguousarray(np.asarray(W2, f).reshape(E * H, D)),
        "b1p": np.ascontiguousarray(np.asarray(b1, f).reshape(-1).reshape(E * H // 128, 128).T),
        "b2": np.ascontiguousarray(np.asarray(b2, f)),
        "wg": np.ascontiguousarray(np.asarray(Wg, f)),
        "wo": np.ascontiguousarray(np.asarray(Wo, f)),
        "bo": np.ascontiguousarray(np.asarray(bo, f).reshape(O, 1)),
    }


def run_on_hw(inputs, trace=False, **kw):
    if "nc" not in _NC_CACHE:
        _NC_CACHE["nc"] = build_nc()
    nc = _NC_CACHE["nc"]
    in_maps = []
    for core in range(NCORES):
        i, c4 = divmod(core, 4)
        x = inputs["x0"] if i == 0 else inputs["x1"]
        in_maps.append(_pack_core_inputs(
            x, inputs["Wg"][i], inputs["W1"][i], inputs["b1"][i],
            inputs["W2"][i], inputs["b2"][i], inputs["Wo"][i], inputs["bo"][i], c4))
    res = run_bass_kernel_spmd(nc, in_maps, core_ids=list(range(NCORES)),
                               trace=trace, **kw)
    outs = []
    for i in range(2):
        outs.append(np.concatenate(
            [res.results[4 * i + c]["outT"].T for c in range(4)], axis=0))
    return (outs[0], outs[1]), res


def kernel(**inputs):
    (o0, o1), _ = run_on_hw(inputs)
    return (o0, o1)



# revision 2
# speedup vs baseline: 2.1015x; 2.1015x over previous
"""Trainium2 Bass kernel for nn_ClassifierGuided (2-modality top-12-of-16 MoE classifier).

Sharding: pure data-parallel over tokens. 2 modalities x 4096 tokens = 8192
tokens; each of the 8 cores owns 1024 tokens of one modality (cores 0-3 ->
modality 0, cores 4-7 -> modality 1) and that modality's full weights.
Dense-eval MoE (all 16 experts computed, sparse gates applied), so no
all-to-all is needed.

Precision plan: the expert MLP runs in fp8 e4m3 via DoubleRow matmuls (two
128-deep contraction planes per instruction at 0.5 cycles/row), which is 4x
the fp32r matmul rate. The MoE branch output is small (~0.08) relative to the
unit-scale residual, so fp8 expert noise lands ~2e-3 in the final output
(tolerance 2e-2). Gating (top-12 selection), the b2/bias terms, the residual,
and the classifier head stay in exact fp32 / fp32r.

Layout: experts processed in quads of 4 (4*192 = 768 h-dims = 6 chunks of
128 = 3 DoubleRow k-pairs). Within a quad the h-dims are host-permuted so
chunks 0-3 are single-expert and chunks 4-5 are half/half mixed at partition
64; the per-chunk gate-broadcast tile [128, 6, NT] is then built with 3
strided DMAs and the gate multiply is one Pool op per chunk (Pool cannot read
PSUM on real TRN2, so ACT drains relu(h+b1) PSUM->SBUF first).

Per-core math (transposed activation layout, d on partitions):
  gating   : logits = x @ Wg -> top-12 mask -> softmax -> gates g [B,16]
  experts  : h_q = relu(x8 @ W1_q + b1); hg_q = fp8(g_q * h_q)
  combine  : moeT = sum_q W2_q^T @ hg_q + b2^T @ gT   (fp8 DR + fp32r close)
  residual : z = relu(moe) + x
  head     : outT = Wo^T @ zT + bo                     (fp32r)
"""
import sys

sys.path.insert(0, "/opt/trn_rl_repo")

import numpy as np
import ml_dtypes

import concourse.bass as bass
import concourse.mybir as mybir
import concourse.tile as tile
from concourse import bacc
from concourse.bass_utils import run_bass_kernel_spmd
from concourse.masks import make_identity

# ---- problem sizes (hardcoded per the harness contract) ----
B = 4096           # tokens per modality
D = 768            # model dim
E = 16             # experts
H = 192            # expert hidden
O = 101            # classifier out
KTOP = 12          # top-k experts
NCORES = 8
BC = B // 4        # 1024 tokens per core
DC = D // 128      # 6 d-chunks
NT = 512           # token tile (matmul moving dim / PSUM bank)
NTILES = BC // NT  # 2
NQ = 4             # expert quads (4 experts each)
HQ = 4 * H         # 768 h-dims per quad = 6 chunks of 128
QCH = HQ // 128    # 6
F32 = mybir.dt.float32
F32R = mybir.dt.float32r
F8 = mybir.dt.float8e4
DR = mybir.MatmulPerfMode.DoubleRow
NEG_BIG = -1.0e30
NPF8 = ml_dtypes.float8_e4m3

_NC_CACHE = {}

# within-quad h-permutation: chunks [e0 0:128 | e1 64:192 | e2 0:128 |
# e3 64:192 | e0 128:192 + e1 0:64 | e2 128:192 + e3 0:64]
_QCHUNKS = [(0, 0, 128), (1, 64, 192), (2, 0, 128), (3, 64, 192),
            (0, 128, 192), (1, 0, 64), (2, 128, 192), (3, 0, 64)]


def _hperm():
    idx = []
    for q in range(NQ):
        for ee, lo, hi in _QCHUNKS:
            idx.extend((4 * q + ee) * H + h for h in range(lo, hi))
    return np.array(idx)


HPERM = _hperm()


def build_nc():
    nc = bacc.Bacc("TRN2", target_bir_lowering=False, debug=False,
                   num_devices=NCORES)

    # ---- DRAM I/O (per-core views; host pre-packs + pre-quantizes) ----
    xT = nc.dram_tensor("xT", [D, BC], F32R, kind="ExternalInput").ap()
    x8d = nc.dram_tensor("x8d", [D, BC], F8, kind="ExternalInput").ap()
    w1p = nc.dram_tensor("w1p", [D, E * H], F8, kind="ExternalInput").ap()
    w2p = nc.dram_tensor("w2p", [E * H, D], F8, kind="ExternalInput").ap()
    b1p = nc.dram_tensor("b1p", [128, E * H // 128], F32, kind="ExternalInput").ap()
    b2 = nc.dram_tensor("b2", [E, D], F32R, kind="ExternalInput").ap()
    wg = nc.dram_tensor("wg", [D, E], F32, kind="ExternalInput").ap()
    wo = nc.dram_tensor("wo", [D, O], F32R, kind="ExternalInput").ap()
    bo = nc.dram_tensor("bo", [O, 1], F32, kind="ExternalInput").ap()
    outT = nc.dram_tensor("outT", [O, BC], F32, kind="ExternalOutput").ap()

    xv = xT.rearrange("(c p) b -> p c b", p=128)
    x8v = x8d.rearrange("(c p) b -> p c b", p=128)
    w1v = w1p.rearrange("(c p) h -> p c h", p=128)
    w2v = w2p.rearrange("(k p) d -> p k d", p=128)
    wgv = wg.rearrange("(c p) e -> p c e", p=128)
    wov = wo.rearrange("(c p) o -> p c o", p=128)

    with tile.TileContext(nc) as tc:
        with tc.tile_pool(name="const", bufs=1) as cpool:
            xsb = cpool.tile([128, DC, BC], F32R)       # x, later z in place
            x8sb = cpool.tile([128, DC, BC], F8)
            w1sb = cpool.tile([128, DC, E * H], F8)
            w2sb = cpool.tile([128, E * H // 128, D], F8)
            b1sb = cpool.tile([128, E * H // 128], F32)
            b2sb = cpool.tile([E, D], F32R)
            wgf = cpool.tile([128, DC, E], F32)
            wosb = cpool.tile([128, DC, O], F32R)
            bosb = cpool.tile([O, 1], F32)
            gT = cpool.tile([E, BC], F32R)              # gates expert-major
            gT8 = cpool.tile([E, BC], F8)
            ident = cpool.tile([128, 128], F32)
            gdram = cpool.tile([E, BC], F8, space="DRAM")

            make_identity(nc, ident[:, :])

            # ---- load schedule: 3 queues (sync=SP, scalar=ACT hwdge,
            # gpsimd=sw dge for gate-broadcast reads) ----
            def piece(i, eng):
                eng.dma_start(out=xsb[:, :, 128 * i:128 * (i + 1)],
                              in_=xv[:, :, 128 * i:128 * (i + 1)])

            nc.sync.dma_start(out=wgf[:, :, :], in_=wgv)
            piece(0, nc.sync)
            piece(1, nc.sync)
            nc.sync.dma_start(out=w1sb[:, :, 0:HQ], in_=w1v[:, :, 0:HQ])
            nc.sync.dma_start(out=b1sb[:, :], in_=b1p)
            piece(4, nc.sync)
            piece(5, nc.sync)
            for q in range(1, NQ):
                nc.sync.dma_start(out=w1sb[:, :, HQ * q:HQ * (q + 1)],
                                  in_=w1v[:, :, HQ * q:HQ * (q + 1)])
            nc.sync.dma_start(out=wosb[:, :, :], in_=wov)
            nc.sync.dma_start(out=b2sb[:, :], in_=b2)
            nc.sync.dma_start(out=bosb[:, :], in_=bo)

            piece(2, nc.scalar)
            piece(3, nc.scalar)
            nc.scalar.dma_start(out=x8sb[:, :, 0:NT], in_=x8v[:, :, 0:NT])
            piece(6, nc.scalar)
            piece(7, nc.scalar)
            for q in range(NQ):
                nc.scalar.dma_start(out=w2sb[:, QCH * q:QCH * (q + 1), :],
                                    in_=w2v[:, QCH * q:QCH * (q + 1), :])
            nc.scalar.dma_start(out=x8sb[:, :, NT:], in_=x8v[:, :, NT:])

            # gate-broadcast tiles: gates round-trip through DRAM in fp8 and
            # are read back with partition-step-0 DMAs. Chunk columns 0-3 are
            # the quad's 4 experts; columns 4-5 are the mixed half/half
            # chunks (expert pairs split at partition 64).
            gb_pre = {}
            ctx_gb = tc.tile_pool(name="gbpool", bufs=3)
            gbpool = ctx_gb.__enter__()

            def load_gb(t, q):
                gb = gbpool.tile([128, QCH, NT], F8, tag="gb", name="gb")
                base = 4 * q * BC + NT * t
                nc.gpsimd.dma_start(
                    out=gb[:, 0:4, :],
                    in_=bass.AP(tensor=gdram.tensor, offset=base,
                                ap=[[0, 128], [BC, 4], [1, NT]]))
                nc.gpsimd.dma_start(
                    out=gb[0:64, 4:6, :],
                    in_=bass.AP(tensor=gdram.tensor, offset=base,
                                ap=[[0, 64], [2 * BC, 2], [1, NT]]))
                nc.gpsimd.dma_start(
                    out=gb[64:128, 4:6, :],
                    in_=bass.AP(tensor=gdram.tensor, offset=base + BC,
                                ap=[[0, 64], [2 * BC, 2], [1, NT]]))
                return gb

            # ---------------- gating pass (128-token subtiles) -------------
            # exact fp32 logits so top-12 selection matches the reference;
            # elementwise chain spread over ACT/DVE/Pool.
            with tc.tile_pool(name="gps", bufs=2, space="PSUM") as gps, \
                 tc.tile_pool(name="gtp", bufs=2, space="PSUM") as gtp, \
                 tc.tile_pool(name="gsb", bufs=3) as gsb:
                for i in range(BC // 128):
                    ts = slice(128 * i, 128 * (i + 1))
                    lg_ps = gps.tile([128, E], F32, tag="lg")
                    for c in range(DC):
                        nc.tensor.matmul(lg_ps[:, :],
                                         xsb[:, c, ts].bitcast(F32),
                                         wgf[:, c, :],
                                         start=(c == 0), stop=(c == DC - 1))
                    lg = gsb.tile([128, E], F32, tag="lg_sb")
                    nc.scalar.activation(lg[:, :], lg_ps[:, :],
                                         mybir.ActivationFunctionType.Identity)
                    # top-8 values, then values 9..16 after masking them out
                    t8a = gsb.tile([128, 8], F32, tag="t8a")
                    nc.vector.max(t8a[:, :], lg[:, :])
                    l2 = gsb.tile([128, E], F32, tag="l2")
                    nc.vector.match_replace(l2[:, :], t8a[:, :], lg[:, :], NEG_BIG)
                    t8b = gsb.tile([128, 8], F32, tag="t8b")
                    nc.vector.max(t8b[:, :], l2[:, :])
                    # softmax over entries >= 12th-largest (t8b[:,3])
                    e16 = gsb.tile([128, E], F32, tag="e16")
                    nc.scalar.activation(e16[:, :], lg[:, :],
                                         mybir.ActivationFunctionType.Exp)
                    em = gsb.tile([128, E], F32, tag="em")
                    ssum = gsb.tile([128, 1], F32, tag="ssum")
                    nc.vector.scalar_tensor_tensor(
                        out=em[:, :], in0=lg[:, :], scalar=t8b[:, 3:4],
                        in1=e16[:, :], op0=mybir.AluOpType.is_ge,
                        op1=mybir.AluOpType.mult, accum_out=ssum[:, :])
                    rinv = gsb.tile([128, 1], F32, tag="rinv")
                    nc.vector.reciprocal(rinv[:, :], ssum[:, :])
                    g = gsb.tile([128, E], F32, tag="g")
                    nc.gpsimd.tensor_scalar_mul(g[:, :], em[:, :], rinv[:, :])
                    # transpose to expert-major gT[16, tokens]
                    gt_ps = gtp.tile([E, 128], F32, tag="gt")
                    nc.tensor.transpose(gt_ps[:, :], g[:, :], ident[:, :])
                    nc.vector.tensor_copy(gT[:, ts], gt_ps[:, :])
                    if i % 4 == 3:
                        # flush this 512-token half's gates to DRAM as fp8 and
                        # prefetch its four gate-broadcast tiles
                        t = i // 4
                        hs = slice(NT * t, NT * (t + 1))
                        nc.vector.tensor_copy(gT8[:, hs], gT[:, hs].bitcast(F32))
                        nc.gpsimd.dma_start(out=gdram[:, hs], in_=gT8[:, hs])
                        for q in range(NQ):
                            gb_pre[(t, q)] = load_gb(t, q)

            # ---------------- main loop ----------------
            # Software pipeline per tile: stage1(q) = W1 DoubleRows -> ACT
            # relu+bias (PSUM->SBUF) -> Pool gate-mult (fp8). W2 DoubleRows of
            # quad q-1 are interleaved into q's m-slots so the in-order PE is
            # never gated on the 2-bank h-PSUM rotation (ACT drain rate).
            with tc.tile_pool(name="moeps", bufs=DC, space="PSUM") as moeps, \
                 tc.tile_pool(name="hps", bufs=2, space="PSUM") as hps, \
                 tc.tile_pool(name="hsb", bufs=4) as hsbpool, \
                 tc.tile_pool(name="hg8", bufs=2) as hg8pool, \
                 tc.tile_pool(name="opool", bufs=2) as opool:
                for t in range(NTILES):
                    ts = slice(NT * t, NT * (t + 1))
                    moe = [moeps.tile([128, NT], F32, tag="moe", name="moe")
                           for _ in range(DC)]
                    hg = [None] * NQ
                    out_ps_box = [None]

                    def w1_chunk(q, m, ts=ts, hg=hg, t=t):
                        # 3 DoubleRow matmuls (contraction 768 = 3 k-pairs),
                        # then relu+bias to SBUF, then Pool gate multiply
                        hcol = HQ * q + 128 * m
                        hp = hps.tile([128, NT], F32, tag="h")
                        for c2 in range(3):
                            nc.tensor.matmul(hp[:, :],
                                             w1sb[:, 2 * c2:2 * c2 + 2,
                                                  hcol:hcol + 128],
                                             x8sb[:, 2 * c2:2 * c2 + 2, ts],
                                             start=(c2 == 0), stop=(c2 == 2),
                                             perf_mode=DR)
                        hs_t = hsbpool.tile([128, NT], F32, tag="hs")
                        k = hcol // 128
                        nc.scalar.activation(hs_t[:, :], hp[:, :],
                                             mybir.ActivationFunctionType.Relu,
                                             bias=b1sb[:, k:k + 1])
                        nc.gpsimd.tensor_tensor(
                            out=hg[q][:, m, :], in0=hs_t[:, :],
                            in1=hg_gb[q][:, m, :],
                            op=mybir.AluOpType.mult)

                    def w2_slot(q, m, moe=moe, hg=hg, t=t):
                        # 3 of quad q's 18 W2 DoubleRows (j2-major order)
                        for idx in range(3 * m, 3 * m + 3):
                            j2, c = divmod(idx, DC)
                            nc.tensor.matmul(moe[c][:, :],
                                             w2sb[:, QCH * q + 2 * j2:
                                                  QCH * q + 2 * j2 + 2,
                                                  128 * c:128 * (c + 1)],
                                             hg[q][:, 2 * j2:2 * j2 + 2, :],
                                             start=(q == 0 and j2 == 0),
                                             stop=False, perf_mode=DR)

                    def finish_chunk(c, moe=moe, ts=ts):
                        # z = relu(moe) + x in one DVE op, overwriting x
                        nc.vector.scalar_tensor_tensor(
                            out=xsb[:, c, ts], in0=moe[c][:, :], scalar=0.0,
                            in1=xsb[:, c, ts].bitcast(F32),
                            op0=mybir.AluOpType.max, op1=mybir.AluOpType.add)

                    def head_chunk(c, ts=ts):
                        if out_ps_box[0] is None:
                            out_ps_box[0] = moeps.tile([O, NT], F32,
                                                       tag="moe", name="out_ps")
                        nc.tensor.matmul(out_ps_box[0][:, :], wosb[:, c, :],
                                         xsb[:, c, ts],
                                         start=(c == 0), stop=(c == DC - 1))

                    hg_gb = [None] * NQ
                    for q in range(NQ):
                        gb = gb_pre.pop((t, q), None)
                        hg_gb[q] = gb if gb is not None else load_gb(t, q)
                        hg[q] = hg8pool.tile([128, QCH, NT], F8, tag="hg",
                                             name="hg")
                        for m in range(QCH):
                            w1_chunk(q, m)
                            if q > 0:
                                w2_slot(q - 1, m)

                    # close: final quad's W2, then per-chunk b2 bias close,
                    # relu+residual drain, trailing head matmuls
                    for c in range(DC):
                        for j2 in range(3):
                            nc.tensor.matmul(moe[c][:, :],
                                             w2sb[:, QCH * 3 + 2 * j2:
                                                  QCH * 3 + 2 * j2 + 2,
                                                  128 * c:128 * (c + 1)],
                                             hg[3][:, 2 * j2:2 * j2 + 2, :],
                                             start=False, stop=False,
                                             perf_mode=DR)
                        nc.tensor.matmul(moe[c][:, :],
                                         b2sb[:, 128 * c:128 * (c + 1)],
                                         gT[:, ts], start=False, stop=True)
                        finish_chunk(c)
                        if c >= 2:
                            head_chunk(c - 2)
                    head_chunk(DC - 2)
                    head_chunk(DC - 1)
                    out_ps = out_ps_box[0]
                    osb = opool.tile([O, NT], F32, tag="osb")
                    nc.scalar.activation(osb[:, :], out_ps[:, :],
                                         mybir.ActivationFunctionType.Identity,
                                         bias=bosb[:, :])
                    nc.sync.dma_start(out=outT[:, ts], in_=osb[:, :])
            ctx_gb.__exit__(None, None, None)

    nc.compile()
    return nc


def _pack_core_inputs(x, Wg, W1, b1, W2, b2, Wo, bo, c4):
    """Per-core input dict for one modality's weights + 1024-token slice."""
    f = np.float32
    tok = slice(BC * c4, BC * (c4 + 1))
    xt = np.ascontiguousarray(np.asarray(x[tok], f).T)
    w1f = np.asarray(W1, f).transpose(1, 0, 2).reshape(D, E * H)[:, HPERM]
    w2f = np.asarray(W2, f).reshape(E * H, D)[HPERM, :]
    b1f = np.asarray(b1, f).reshape(E * H)[HPERM]
    return {
        "xT": xt,
        "x8d": xt.astype(NPF8),
        "w1p": np.ascontiguousarray(w1f.astype(NPF8)),
        "w2p": np.ascontiguousarray(w2f.astype(NPF8)),
        "b1p": np.ascontiguousarray(b1f.reshape(E * H // 128, 128).T),
        "b2": np.ascontiguousarray(np.asarray(b2, f)),
        "wg": np.ascontiguousarray(np.asarray(Wg, f)),
        "wo": np.ascontiguousarray(np.asarray(Wo, f)),
        "bo": np.ascontiguousarray(np.asarray(bo, f).reshape(O, 1)),
    }


def run_on_hw(inputs, trace=False, **kw):
    if "nc" not in _NC_CACHE:
        _NC_CACHE["nc"] = build_nc()
    nc = _NC_CACHE["nc"]
    in_maps = []
    for core in range(NCORES):
        i, c4 = divmod(core, 4)
        x = inputs["x0"] if i == 0 else inputs["x1"]
        in_maps.append(_pack_core_inputs(
            x, inputs["Wg"][i], inputs["W1"][i], inputs["b1"][i],
            inputs["W2"][i], inputs["b2"][i], inputs["Wo"][i], inputs["bo"][i], c4))
    res = run_bass_kernel_spmd(nc, in_maps, core_ids=list(range(NCORES)),
                               trace=trace, **kw)
    outs = []
    for i in range(2):
        outs.append(np.concatenate(
            [res.results[4 * i + c]["outT"].T for c in range(4)], axis=0))
    return (outs[0], outs[1]), res


def kernel(**inputs):
    (o0, o1), _ = run_on_hw(inputs)
    return (o0, o1)


# revision 10
# speedup vs baseline: 2.2256x; 1.0591x over previous
"""Trainium2 Bass kernel for nn_ClassifierGuided (2-modality top-12-of-16 MoE classifier).

Sharding: pure data-parallel over tokens. 2 modalities x 4096 tokens = 8192
tokens; each of the 8 cores owns 1024 tokens of one modality (cores 0-3 ->
modality 0, cores 4-7 -> modality 1) and that modality's full weights.
Dense-eval MoE (all 16 experts computed, sparse gates applied), so no
all-to-all is needed.

Precision: expert MLP + b2 close in fp8 e4m3 via DoubleRow matmuls (two
128-deep contraction planes per instruction at 0.5 cycles/row = 4x the fp32r
rate). Gating, residual and head run in bf16; top-12 selection flips are rare
near-ties with negligible gate deltas. Measured end-to-end error ~4e-3
against the fp32 reference (tolerance 2e-2).

Layout: 24 h-chunks of 128. Chunks 0-15 are single-expert ("pure": expert e
keeps h[0:128] if e even else h[64:192]); chunks 16-23 are half/half mixed
(expert 2j h[128:192] on partitions 0:64, expert 2j+1 h[0:64] on 64:128).
Gates stream to DRAM as fp8 and come back as a per-chunk broadcast table
[128, 24, NT] in 3 strided DMAs, so the gate multiply is one Pool op per
chunk. Quad q = experts 4q..4q+3 = chunks [4q..4q+4) + [16+2q, 17+2q].

Pipeline: W2 DoubleRows of quad q-1 interleave with W1 chunks of quad q so
the in-order PE never stalls on the 2-bank h-PSUM rotation; relu+bias splits
between ACT (4/quad) and DVE (2/quad); tile 1's first quad runs inside tile
0's close; gating for the second token half is issued mid-pipeline.
"""
import sys

sys.path.insert(0, "/opt/trn_rl_repo")

import numpy as np
import ml_dtypes

import concourse.bass as bass
import concourse.mybir as mybir
import concourse.tile as tile
from concourse import bacc
from concourse.bass_utils import run_bass_kernel_spmd
from concourse.masks import make_identity

# ---- problem sizes (hardcoded per the harness contract) ----
B = 4096           # tokens per modality
D = 768            # model dim
E = 16             # experts
H = 192            # expert hidden
O = 101            # classifier out
KTOP = 12          # top-k experts
NCORES = 8
BC = B // 4        # 1024 tokens per core
DC = D // 128      # 6 d-chunks
NT = 512           # token tile (matmul moving dim / PSUM bank)
NTILES = BC // NT  # 2
NQ = 4             # expert quads
NCH = E * H // 128  # 24 h-chunks
F32 = mybir.dt.float32
BF16 = mybir.dt.bfloat16
F8 = mybir.dt.float8e4
DR = mybir.MatmulPerfMode.DoubleRow
NEG_BIG = -1.0e30
NPF8 = ml_dtypes.float8_e4m3
NPBF = ml_dtypes.bfloat16

_NC_CACHE = {}


def _hperm():
    """Global h-permutation: 16 pure chunks then 8 mixed chunks."""
    idx = []
    for e in range(E):
        lo = 0 if e % 2 == 0 else 64
        idx.extend(e * H + h for h in range(lo, lo + 128))
    for j in range(8):
        idx.extend((2 * j) * H + h for h in range(128, 192))
        idx.extend((2 * j + 1) * H + h for h in range(0, 64))
    return np.array(idx)


HPERM = _hperm()
# quad q covers chunks [4q, 4q+1, 4q+2, 4q+3, 16+2q, 17+2q]
QCHUNK = [[4 * q, 4 * q + 1, 4 * q + 2, 4 * q + 3, 16 + 2 * q, 17 + 2 * q]
          for q in range(NQ)]


def build_nc():
    nc = bacc.Bacc("TRN2", target_bir_lowering=False, debug=False,
                   num_devices=NCORES)

    # ---- DRAM I/O (per-core views; host pre-packs + pre-quantizes) ----
    xbf = nc.dram_tensor("xbf", [D, BC], BF16, kind="ExternalInput").ap()
    x8d = nc.dram_tensor("x8d", [D, BC], F8, kind="ExternalInput").ap()
    w1p = nc.dram_tensor("w1p", [D, E * H], F8, kind="ExternalInput").ap()
    w2p = nc.dram_tensor("w2p", [E * H, D], F8, kind="ExternalInput").ap()
    b1p = nc.dram_tensor("b1p", [128, NCH], F32, kind="ExternalInput").ap()
    b28 = nc.dram_tensor("b28", [8, 2 * D], F8, kind="ExternalInput").ap()
    wg = nc.dram_tensor("wg", [D, E], BF16, kind="ExternalInput").ap()
    wo = nc.dram_tensor("wo", [D, O], BF16, kind="ExternalInput").ap()
    bo = nc.dram_tensor("bo", [O, 1], F32, kind="ExternalInput").ap()
    outT = nc.dram_tensor("outT", [O, BC], F32, kind="ExternalOutput").ap()

    xv = xbf.rearrange("(c p) b -> p c b", p=128)
    x8v = x8d.rearrange("(c p) b -> p c b", p=128)
    w1v = w1p.rearrange("(c p) h -> p c h", p=128)
    w2v = w2p.rearrange("(k p) d -> p k d", p=128)
    wgv = wg.rearrange("(c p) e -> p c e", p=128)
    wov = wo.rearrange("(c p) o -> p c o", p=128)

    with tile.TileContext(nc) as tc:
        with tc.tile_pool(name="const", bufs=1) as cpool:
            xsb = cpool.tile([128, DC, BC], BF16)       # x, later z in place
            x8sb = cpool.tile([128, DC, BC], F8)
            w1sb = cpool.tile([128, DC, E * H], F8)
            w2sb = cpool.tile([128, NCH, D], F8)
            b1sb = cpool.tile([128, NCH], F32)
            b2sb = cpool.tile([8, 2, D], F8)
            wgsb = cpool.tile([128, DC, E], BF16)
            wosb = cpool.tile([128, DC, O], BF16)
            bosb = cpool.tile([O, 1], F32)
            gT8b = cpool.tile([8, 2, BC], F8)           # expert e = p + 8*blk
            zeros = cpool.tile([128, NT], F32)
            ident = cpool.tile([128, 128], F32)
            gdram = cpool.tile([E, BC], F8, space="DRAM")

            make_identity(nc, ident[:, :])
            nc.vector.memset(zeros[:, :], 0.0)

            # ---- load schedule (SP + ACT hwdge queues; gb reads on both
            # SP and Pool). Order is critical: engines are in-order, and a
            # queued DMA blocks later compute on the same engine. ----
            def xq(i, eng):   # quarter of xbf (256 tokens, innermost 512B)
                eng.dma_start(out=xsb[:, :, 256 * i:256 * (i + 1)],
                              in_=xv[:, :, 256 * i:256 * (i + 1)])

            HQ = 4 * H
            nc.sync.dma_start(out=wgsb[:, :, :], in_=wgv)
            xq(0, nc.sync)
            xq(1, nc.sync)
            nc.sync.dma_start(out=x8sb[:, :, 0:NT], in_=x8v[:, :, 0:NT])
            nc.sync.dma_start(out=w1sb[:, :, 0:HQ], in_=w1v[:, :, 0:HQ])
            nc.sync.dma_start(out=b1sb[:, :], in_=b1p)
            nc.sync.dma_start(out=w1sb[:, :, HQ:2 * HQ],
                              in_=w1v[:, :, HQ:2 * HQ])
            xq(2, nc.sync)
            xq(3, nc.sync)
            nc.sync.dma_start(out=w1sb[:, :, 2 * HQ:3 * HQ],
                              in_=w1v[:, :, 2 * HQ:3 * HQ])
            nc.sync.dma_start(out=w1sb[:, :, 3 * HQ:4 * HQ],
                              in_=w1v[:, :, 3 * HQ:4 * HQ])
            nc.sync.dma_start(out=x8sb[:, :, NT:], in_=x8v[:, :, NT:])
            nc.sync.dma_start(out=wosb[:, :, :], in_=wov)
            nc.sync.dma_start(out=b2sb[:, :, :],
                              in_=b28.rearrange("p (k d) -> p k d", k=2))
            nc.sync.dma_start(out=bosb[:, :], in_=bo)

            # ACT queue: only w2 quad0 before gating compute; the other w2
            # quads are threaded into per-quad ACT slack mid-pipeline
            nc.scalar.dma_start(out=w2sb[:, 0:4, :], in_=w2v[:, 0:4, :])

            def load_w2q(q, eng=None):
                # quad q's W2 chunk-rows: pure 4q:4q+4 and mixed 16+2q:18+2q
                (eng or nc.scalar).dma_start(
                    out=w2sb[:, 4 * q:4 * q + 4, :],
                    in_=w2v[:, 4 * q:4 * q + 4, :])

            def load_w2m(eng):  # all mixed chunk rows 16:24
                eng.dma_start(out=w2sb[:, 16:24, :], in_=w2v[:, 16:24, :])

            # gate-broadcast table reads: fp8 gates round-trip through DRAM,
            # partition-step-0 reads build gball [128, 24, NT]
            gb_tiles = {}
            ctx_gb = tc.tile_pool(name="gball", bufs=2)
            gbpool = ctx_gb.__enter__()

            def gb_alloc(t):
                gb_tiles[t] = gbpool.tile([128, NCH, NT], F8, tag="gb",
                                          name="gball")
                return gb_tiles[t]

            def gb_pure(t, q0, q1, eng):
                # pure chunk cols q0*4 : q1*4 (rows = experts, stride BC)
                gb = gb_tiles[t]
                eng.dma_start(
                    out=gb[:, 4 * q0:4 * q1, :],
                    in_=bass.AP(tensor=gdram.tensor,
                                offset=4 * q0 * BC + NT * t,
                                ap=[[0, 128], [BC, 4 * (q1 - q0)], [1, NT]]))

            def gb_mixed(t, eng):
                # mixed cols 16:24: even expert rows on partitions 0:64,
                # odd expert rows on partitions 64:128
                gb = gb_tiles[t]
                eng.dma_start(
                    out=gb[0:64, 16:24, :],
                    in_=bass.AP(tensor=gdram.tensor, offset=NT * t,
                                ap=[[0, 64], [2 * BC, 8], [1, NT]]))
                eng.dma_start(
                    out=gb[64:128, 16:24, :],
                    in_=bass.AP(tensor=gdram.tensor, offset=BC + NT * t,
                                ap=[[0, 64], [2 * BC, 8], [1, NT]]))

            # ---------------- gating (bf16 logits, exact-enough top-12) ----
            gate_ctxs = [tc.tile_pool(name="gsb", bufs=3)]
            gsb = gate_ctxs[0].__enter__()

            def gating_half(hf, hps):
                for i in range(4 * hf, 4 * hf + 4):
                    ts = slice(128 * i, 128 * (i + 1))
                    lg_ps = hps.tile([128, E], F32, tag="h", name="lg_ps")
                    for c in range(DC):
                        nc.tensor.matmul(lg_ps[:, :], xsb[:, c, ts],
                                         wgsb[:, c, :],
                                         start=(c == 0), stop=(c == DC - 1))
                    lg = gsb.tile([128, E], F32, tag="lg_sb")
                    nc.scalar.activation(lg[:, :], lg_ps[:, :],
                                         mybir.ActivationFunctionType.Identity)
                    t8a = gsb.tile([128, 8], F32, tag="t8a")
                    nc.vector.max(t8a[:, :], lg[:, :])
                    l2 = gsb.tile([128, E], F32, tag="l2")
                    nc.vector.match_replace(l2[:, :], t8a[:, :], lg[:, :],
                                            NEG_BIG)
                    t8b = gsb.tile([128, 8], F32, tag="t8b")
                    nc.vector.max(t8b[:, :], l2[:, :])
                    e16 = gsb.tile([128, E], F32, tag="e16")
                    nc.scalar.activation(e16[:, :], lg[:, :],
                                         mybir.ActivationFunctionType.Exp)
                    em = gsb.tile([128, E], F32, tag="em")
                    ssum = gsb.tile([128, 1], F32, tag="ssum")
                    nc.vector.scalar_tensor_tensor(
                        out=em[:, :], in0=lg[:, :], scalar=t8b[:, 3:4],
                        in1=e16[:, :], op0=mybir.AluOpType.is_ge,
                        op1=mybir.AluOpType.mult, accum_out=ssum[:, :])
                    rinv = gsb.tile([128, 1], F32, tag="rinv")
                    nc.vector.reciprocal(rinv[:, :], ssum[:, :])
                    g = gsb.tile([128, E], F32, tag="g")
                    nc.gpsimd.tensor_scalar_mul(g[:, :], em[:, :], rinv[:, :])
                    gt_ps = hps.tile([E, 128], F32, tag="h", name="gt_ps")
                    nc.tensor.transpose(gt_ps[:, :], g[:, :], ident[:, :])
                    # expert e -> gT8b[e % 8, e // 8]; one copy ACT, one DVE
                    nc.scalar.activation(gT8b[:, 0, ts], gt_ps[0:8, :],
                                         mybir.ActivationFunctionType.Identity)
                    nc.vector.tensor_copy(gT8b[:, 1, ts], gt_ps[8:16, :])
                hs = slice(NT * hf, NT * (hf + 1))
                # flush to DRAM rows e = p + 8*blk
                nc.gpsimd.dma_start(
                    out=bass.AP(tensor=gdram.tensor, offset=NT * hf,
                                ap=[[BC, 8], [8 * BC, 2], [1, NT]]),
                    in_=gT8b[:, :, hs])

            # ---------------- main pipeline ----------------
            with tc.tile_pool(name="moeps", bufs=DC, space="PSUM") as moeps, \
                 tc.tile_pool(name="hps", bufs=2, space="PSUM") as hps, \
                 tc.tile_pool(name="hsb", bufs=4) as hsbpool, \
                 tc.tile_pool(name="hg8", bufs=2) as hg8pool, \
                 tc.tile_pool(name="opool", bufs=2) as opool:

                hg_tiles = {}
                moe_tiles = {}
                out_ps_box = {}

                def w1_chunk(t, q, m):
                    # 3 W1 DoubleRows -> relu+bias (ACT or DVE) -> Pool gate
                    # multiply into hg[(t,q)][:, m, :] (fp8)
                    ts = slice(NT * t, NT * (t + 1))
                    k = QCHUNK[q][m]
                    hp = hps.tile([128, NT], F32, tag="h", name="h")
                    for c2 in range(3):
                        nc.tensor.matmul(hp[:, :],
                                         w1sb[:, 2 * c2:2 * c2 + 2,
                                              128 * k:128 * (k + 1)],
                                         x8sb[:, 2 * c2:2 * c2 + 2, ts],
                                         start=(c2 == 0), stop=(c2 == 2),
                                         perf_mode=DR)
                    hs_t = hsbpool.tile([128, NT], F32, tag="hs")
                    if m in (1, 4):   # 2 of 6 relus per quad go to DVE
                        nc.vector.scalar_tensor_tensor(
                            out=hs_t[:, :], in0=hp[:, :],
                            scalar=b1sb[:, k:k + 1], in1=zeros[:, :],
                            op0=mybir.AluOpType.add, op1=mybir.AluOpType.max)
                    else:
                        nc.scalar.activation(hs_t[:, :], hp[:, :],
                                             mybir.ActivationFunctionType.Relu,
                                             bias=b1sb[:, k:k + 1])
                    nc.gpsimd.tensor_tensor(
                        out=hg_tiles[(t, q)][:, m, :], in0=hs_t[:, :],
                        in1=gb_tiles[t][:, k, :], op=mybir.AluOpType.mult)

                def w2_slot(t, q, m):
                    # 3 of quad q's 18 W2 DoubleRows (pair-major order)
                    moe = moe_tiles[t]
                    for idx in range(3 * m, 3 * m + 3):
                        j, c = divmod(idx, DC)
                        kp = 4 * q + 2 * j if j < 2 else 16 + 2 * q
                        nc.tensor.matmul(moe[c][:, :],
                                         w2sb[:, kp:kp + 2,
                                              128 * c:128 * (c + 1)],
                                         hg_tiles[(t, q)][:, 2 * j:2 * j + 2, :],
                                         start=(q == 0 and j == 0),
                                         stop=False, perf_mode=DR)

                def stage1(t, q, prev=None, post_m=None):
                    gb = gb_tiles.get(t)
                    hg_tiles[(t, q)] = hg8pool.tile([128, DC, NT], F8,
                                                    tag="hg", name="hg")
                    for m in range(DC):
                        w1_chunk(t, q, m)
                        if prev is not None:
                            w2_slot(t, prev[1], m)
                        if post_m is not None and m in post_m:
                            post_m[m]()

                def finish_chunk(t, c):
                    # z = relu(moe) + x in one DVE op (bf16 out, in place)
                    ts = slice(NT * t, NT * (t + 1))
                    moe = moe_tiles[t]
                    nc.vector.scalar_tensor_tensor(
                        out=xsb[:, c, ts], in0=moe[c][:, :], scalar=0.0,
                        in1=xsb[:, c, ts], op0=mybir.AluOpType.max,
                        op1=mybir.AluOpType.add)

                def head_chunk(t, c):
                    ts = slice(NT * t, NT * (t + 1))
                    if t not in out_ps_box:
                        out_ps_box[t] = moeps.tile([O, NT], F32, tag="moe",
                                                   name="out_ps")
                    nc.tensor.matmul(out_ps_box[t][:, :], wosb[:, c, :],
                                     xsb[:, c, ts],
                                     start=(c == 0), stop=(c == DC - 1))

                def close_tile(t, interleave_next=False):
                    ts = slice(NT * t, NT * (t + 1))
                    moe = moe_tiles[t]
                    for c in range(DC):
                        if interleave_next and c == 0:
                            gb_alloc(t + 1)
                            gb_pure(t + 1, 0, 4, nc.sync)
                            gb_mixed(t + 1, nc.sync)
                            hg_tiles[(t + 1, 0)] = hg8pool.tile(
                                [128, DC, NT], F8, tag="hg", name="hg")
                        for j in range(3):   # quad 3's W2, c-major
                            kp = 12 + 2 * j if j < 2 else 22
                            nc.tensor.matmul(moe[c][:, :],
                                             w2sb[:, kp:kp + 2,
                                                  128 * c:128 * (c + 1)],
                                             hg_tiles[(t, 3)][:, 2 * j:2 * j + 2, :],
                                             start=False, stop=False,
                                             perf_mode=DR)
                        # b2 bias close: fp8 DoubleRow over the 16 experts
                        nc.tensor.matmul(moe[c][:, :],
                                         b2sb[:, :, 128 * c:128 * (c + 1)],
                                         gT8b[:, :, ts],
                                         start=False, stop=True, perf_mode=DR)
                        finish_chunk(t, c)
                        if c >= 1:
                            head_chunk(t, c - 1)
                        if interleave_next:
                            w1_chunk(t + 1, 0, c)
                    head_chunk(t, DC - 1)
                    osb = opool.tile([O, NT], F32, tag="osb")
                    nc.scalar.activation(osb[:, :], out_ps_box[t][:, :],
                                         mybir.ActivationFunctionType.Identity,
                                         bias=bosb[:, :])
                    nc.sync.dma_start(out=outT[:, ts], in_=osb[:, :])

                # ---- driver ----
                gating_half(0, hps)
                load_w2m(nc.scalar)
                gb_alloc(0)
                gb_pure(0, 0, 1, nc.gpsimd)
                gb_mixed(0, nc.gpsimd)
                gb_pure(0, 1, 4, nc.gpsimd)

                moe_tiles[0] = [moeps.tile([128, NT], F32, tag="moe",
                                           name="moe") for _ in range(DC)]
                stage1(0, 0)
                stage1(0, 1, prev=(0, 0),
                       post_m={1: lambda: load_w2q(1)})
                stage1(0, 2, prev=(0, 1),
                       post_m={1: lambda: load_w2q(2)})
                # second-half gating issued mid-pipeline (engines in-order;
                # its PE/ACT/DVE slices fit the per-quad slack here)
                gating_half(1, hps)
                stage1(0, 3, prev=(0, 2), post_m={1: lambda: load_w2q(3)})
                close_tile(0, interleave_next=True)

                moe_tiles[1] = [moeps.tile([128, NT], F32, tag="moe",
                                           name="moe") for _ in range(DC)]
                stage1(1, 1, prev=(1, 0))
                stage1(1, 2, prev=(1, 1))
                stage1(1, 3, prev=(1, 2))
                close_tile(1)

            gate_ctxs[0].__exit__(None, None, None)
            ctx_gb.__exit__(None, None, None)

    nc.compile()
    return nc


def _pack_core_inputs(x, Wg, W1, b1, W2, b2, Wo, bo, c4):
    """Per-core input dict for one modality's weights + 1024-token slice."""
    f = np.float32
    tok = slice(BC * c4, BC * (c4 + 1))
    xt = np.ascontiguousarray(np.asarray(x[tok], f).T)
    w1f = np.asarray(W1, f).transpose(1, 0, 2).reshape(D, E * H)[:, HPERM]
    w2f = np.asarray(W2, f).reshape(E * H, D)[HPERM, :]
    b1f = np.asarray(b1, f).reshape(E * H)[HPERM]
    b2f = np.asarray(b2, f)          # [16, D]; row e -> [e % 8, (e//8)*D]
    b28 = np.concatenate([b2f[0:8], b2f[8:16]], axis=1)
    return {
        "xbf": xt.astype(NPBF),
        "x8d": xt.astype(NPF8),
        "w1p": np.ascontiguousarray(w1f.astype(NPF8)),
        "w2p": np.ascontiguousarray(w2f.astype(NPF8)),
        "b1p": np.ascontiguousarray(b1f.reshape(NCH, 128).T),
        "b28": np.ascontiguousarray(b28.astype(NPF8)),
        "wg": np.ascontiguousarray(np.asarray(Wg, f).astype(NPBF)),
        "wo": np.ascontiguousarray(np.asarray(Wo, f).astype(NPBF)),
        "bo": np.ascontiguousarray(np.asarray(bo, f).reshape(O, 1)),
    }


def run_on_hw(inputs, trace=False, **kw):
    if "nc" not in _NC_CACHE:
        _NC_CACHE["nc"] = build_nc()
    nc = _NC_CACHE["nc"]
    in_maps = []
    for core in range(NCORES):
        i, c4 = divmod(core, 4)
        x = inputs["x0"] if i == 0 else inputs["x1"]
        in_maps.append(_pack_core_inputs(
            x, inputs["Wg"][i], inputs["W1"][i], inputs["b1"][i],
            inputs["W2"][i], inputs["b2"][i], inputs["Wo"][i], inputs["bo"][i], c4))
    res = run_bass_kernel_spmd(nc, in_maps, core_ids=list(range(NCORES)),
                               trace=trace, **kw)
    outs = []
    for i in range(2):
        outs.append(np.concatenate(
            [res.results[4 * i + c]["outT"].T for c in range(4)], axis=0))
    return (outs[0], outs[1]), res


def kernel(**inputs):
    (o0, o1), _ = run_on_hw(inputs)
    return (o0, o1)


# revision 16
# speedup vs baseline: 2.4981x; 1.1224x over previous
"""Trainium2 Bass kernel for nn_ClassifierGuided (2-modality top-12-of-16 MoE classifier).

Sharding: pure data-parallel over tokens. 2 modalities x 4096 tokens = 8192
tokens; each of the 8 cores owns 1024 tokens of one modality (cores 0-3 ->
modality 0, cores 4-7 -> modality 1) and that modality's full weights.
Dense-eval MoE (all 16 experts computed, sparse gates applied), so no
all-to-all is needed.

Precision: expert MLP + b2 close in fp8 e4m3 via DoubleRow matmuls (two
128-deep contraction planes per instruction at 0.5 cycles/row = 4x the fp32r
rate). Gating, residual and head run in bf16; top-12 selection flips are rare
near-ties with negligible gate deltas. Measured end-to-end error ~4e-3
against the fp32 reference (tolerance 2e-2).

Layout: 24 h-chunks of 128. Chunks 0-15 are single-expert ("pure": expert e
keeps h[0:128] if e even else h[64:192]); chunks 16-23 are half/half mixed
(expert 2j h[128:192] on partitions 0:64, expert 2j+1 h[0:64] on 64:128).
Gates stream to DRAM as fp8 and come back as a per-chunk broadcast table
[128, 24, NT] in 3 strided DMAs, so the gate multiply is one Pool op per
chunk. Quad q = experts 4q..4q+3 = chunks [4q..4q+4) + [16+2q, 17+2q].

Pipeline: W2 DoubleRows of quad q-1 interleave with W1 chunks of quad q so
the in-order PE never stalls on the 2-bank h-PSUM rotation; relu+bias splits
between ACT (4/quad) and DVE (2/quad); tile 1's first quad runs inside tile
0's close; gating for the second token half is issued mid-pipeline.
"""
import sys

sys.path.insert(0, "/opt/trn_rl_repo")

import numpy as np
import ml_dtypes

import concourse.bass as bass
import concourse.mybir as mybir
import concourse.tile as tile
from concourse import bacc
from concourse.bass_utils import run_bass_kernel_spmd
from concourse.masks import make_identity

# ---- problem sizes (hardcoded per the harness contract) ----
B = 4096           # tokens per modality
D = 768            # model dim
E = 16             # experts
H = 192            # expert hidden
O = 101            # classifier out
KTOP = 12          # top-k experts
NCORES = 8
BC = B // 4        # 1024 tokens per core
DC = D // 128      # 6 d-chunks
NT = 512           # token tile (matmul moving dim / PSUM bank)
NTILES = BC // NT  # 2
NQ = 4             # expert quads
NCH = E * H // 128  # 24 h-chunks
F32 = mybir.dt.float32
BF16 = mybir.dt.bfloat16
F8 = mybir.dt.float8e4
DR = mybir.MatmulPerfMode.DoubleRow
NEG_BIG = -1.0e30
NPF8 = ml_dtypes.float8_e4m3
NPBF = ml_dtypes.bfloat16

_NC_CACHE = {}


def _hperm():
    """Global h-permutation: 16 pure chunks then 8 mixed chunks."""
    idx = []
    for e in range(E):
        lo = 0 if e % 2 == 0 else 64
        idx.extend(e * H + h for h in range(lo, lo + 128))
    for j in range(8):
        idx.extend((2 * j) * H + h for h in range(128, 192))
        idx.extend((2 * j + 1) * H + h for h in range(0, 64))
    return np.array(idx)


HPERM = _hperm()
# quad q covers chunks [4q, 4q+1, 4q+2, 4q+3, 16+2q, 17+2q]
QCHUNK = [[4 * q, 4 * q + 1, 4 * q + 2, 4 * q + 3, 16 + 2 * q, 17 + 2 * q]
          for q in range(NQ)]


def build_nc():
    nc = bacc.Bacc("TRN2", target_bir_lowering=False, debug=False,
                   num_devices=NCORES)

    # ---- DRAM I/O (per-core views; host pre-packs + pre-quantizes) ----
    xbf = nc.dram_tensor("xbf", [D, BC], BF16, kind="ExternalInput").ap()
    x8d = nc.dram_tensor("x8d", [D, BC], F8, kind="ExternalInput").ap()
    w1p = nc.dram_tensor("w1p", [D, E * H], F8, kind="ExternalInput").ap()
    w2p = nc.dram_tensor("w2p", [E * H, D], F8, kind="ExternalInput").ap()
    b1p = nc.dram_tensor("b1p", [128, NCH], F32, kind="ExternalInput").ap()
    b28 = nc.dram_tensor("b28", [8, 2 * D], F8, kind="ExternalInput").ap()
    wg = nc.dram_tensor("wg", [D, E], BF16, kind="ExternalInput").ap()
    wo = nc.dram_tensor("wo", [D, O], BF16, kind="ExternalInput").ap()
    bo = nc.dram_tensor("bo", [O, 1], F32, kind="ExternalInput").ap()
    outT = nc.dram_tensor("outT", [O, BC], F32, kind="ExternalOutput").ap()

    xv = xbf.rearrange("(c p) b -> p c b", p=128)
    x8v = x8d.rearrange("(c p) b -> p c b", p=128)
    w1v = w1p.rearrange("(c p) h -> p c h", p=128)
    w2v = w2p.rearrange("(k p) d -> p k d", p=128)
    wgv = wg.rearrange("(c p) e -> p c e", p=128)
    wov = wo.rearrange("(c p) o -> p c o", p=128)

    with tile.TileContext(nc) as tc:
        with tc.tile_pool(name="const", bufs=1) as cpool:
            xsb = cpool.tile([128, DC, BC], BF16)       # x, later z in place
            x8sb = cpool.tile([128, DC, BC], F8)
            w1sb = cpool.tile([128, DC, E * H], F8)
            w2sb = cpool.tile([128, NCH, D], F8)
            b1sb = cpool.tile([128, NCH], F32)
            b2sb = cpool.tile([8, 2, D], F8)
            wgsb = cpool.tile([128, DC, E], BF16)
            wosb = cpool.tile([128, DC, O], BF16)
            bosb = cpool.tile([O, 1], F32)
            zeros = cpool.tile([128, NT], F32)
            gdram = cpool.tile([E, BC], F8, space="DRAM")

            nc.vector.memset(zeros[:, :], 0.0)

            # ---- load schedule (SP + ACT hwdge queues; gb reads on both
            # SP and Pool). Order is critical: engines are in-order, and a
            # queued DMA blocks later compute on the same engine. ----
            def xq(i, eng):   # quarter of xbf (256 tokens, innermost 512B)
                eng.dma_start(out=xsb[:, :, 256 * i:256 * (i + 1)],
                              in_=xv[:, :, 256 * i:256 * (i + 1)])

            HQ = 4 * H
            nc.sync.dma_start(out=wgsb[:, :, :], in_=wgv)
            xq(0, nc.sync)
            xq(1, nc.sync)
            nc.sync.dma_start(out=x8sb[:, :, 0:NT], in_=x8v[:, :, 0:NT])
            nc.sync.dma_start(out=w1sb[:, :, 0:HQ], in_=w1v[:, :, 0:HQ])
            nc.sync.dma_start(out=b1sb[:, :], in_=b1p)
            nc.sync.dma_start(out=w1sb[:, :, HQ:2 * HQ],
                              in_=w1v[:, :, HQ:2 * HQ])
            xq(2, nc.sync)
            xq(3, nc.sync)
            nc.sync.dma_start(out=w1sb[:, :, 2 * HQ:3 * HQ],
                              in_=w1v[:, :, 2 * HQ:3 * HQ])
            nc.sync.dma_start(out=w1sb[:, :, 3 * HQ:4 * HQ],
                              in_=w1v[:, :, 3 * HQ:4 * HQ])
            nc.sync.dma_start(out=x8sb[:, :, NT:], in_=x8v[:, :, NT:])
            nc.sync.dma_start(out=wosb[:, :, :], in_=wov)
            nc.sync.dma_start(out=b2sb[:, :, :],
                              in_=b28.rearrange("p (k d) -> p k d", k=2))
            nc.sync.dma_start(out=bosb[:, :], in_=bo)

            # ACT queue: only w2 quad0 before gating compute; the other w2
            # quads are threaded into per-quad ACT slack mid-pipeline
            nc.scalar.dma_start(out=w2sb[:, 0:4, :], in_=w2v[:, 0:4, :])

            def load_w2q(q, eng=None):
                # quad q's W2 chunk-rows: pure 4q:4q+4 and mixed 16+2q:18+2q
                (eng or nc.scalar).dma_start(
                    out=w2sb[:, 4 * q:4 * q + 4, :],
                    in_=w2v[:, 4 * q:4 * q + 4, :])

            def load_w2m(eng):  # all mixed chunk rows 16:24
                eng.dma_start(out=w2sb[:, 16:24, :], in_=w2v[:, 16:24, :])

            # gate-broadcast table reads: fp8 gates round-trip through DRAM,
            # partition-step-0 reads build gball [128, 24, NT]
            gb_tiles = {}
            ctx_gb = tc.tile_pool(name="gball", bufs=2)
            gbpool = ctx_gb.__enter__()

            def gb_alloc(t):
                gb_tiles[t] = gbpool.tile([128, NCH, NT], F8, tag="gb",
                                          name="gball")
                return gb_tiles[t]

            def gb_pure(t, q0, q1, eng):
                # pure chunk cols q0*4 : q1*4 (rows = experts, stride BC)
                gb = gb_tiles[t]
                eng.dma_start(
                    out=gb[:, 4 * q0:4 * q1, :],
                    in_=bass.AP(tensor=gdram.tensor,
                                offset=4 * q0 * BC + NT * t,
                                ap=[[0, 128], [BC, 4 * (q1 - q0)], [1, NT]]))

            def gb_mixed(t, eng):
                # mixed cols 16:24: even expert rows on partitions 0:64,
                # odd expert rows on partitions 64:128
                gb = gb_tiles[t]
                eng.dma_start(
                    out=gb[0:64, 16:24, :],
                    in_=bass.AP(tensor=gdram.tensor, offset=NT * t,
                                ap=[[0, 64], [2 * BC, 8], [1, NT]]))
                eng.dma_start(
                    out=gb[64:128, 16:24, :],
                    in_=bass.AP(tensor=gdram.tensor, offset=BC + NT * t,
                                ap=[[0, 64], [2 * BC, 8], [1, NT]]))

            # ---------------- gating (bf16 logits, exact-enough top-12) ----
            gate_ctxs = [tc.tile_pool(name="gsb", bufs=3)]
            gsb = gate_ctxs[0].__enter__()

            def gating_half(hf, hps):
                for i in range(4 * hf, 4 * hf + 4):
                    ts = slice(128 * i, 128 * (i + 1))
                    lg_ps = hps.tile([128, E], F32, tag="h", name="lg_ps")
                    for c in range(DC):
                        nc.tensor.matmul(lg_ps[:, :], xsb[:, c, ts],
                                         wgsb[:, c, :],
                                         start=(c == 0), stop=(c == DC - 1))
                    lg = gsb.tile([128, E], F32, tag="lg_sb")
                    nc.vector.tensor_copy(lg[:, :], lg_ps[:, :])
                    # exp on ACT runs concurrently with the DVE top-k chain
                    e16 = gsb.tile([128, E], F32, tag="e16")
                    nc.scalar.activation(e16[:, :], lg[:, :],
                                         mybir.ActivationFunctionType.Exp)
                    t8a = gsb.tile([128, 8], F32, tag="t8a")
                    nc.vector.max(t8a[:, :], lg[:, :])
                    l2 = gsb.tile([128, E], F32, tag="l2")
                    nc.vector.match_replace(l2[:, :], t8a[:, :], lg[:, :],
                                            NEG_BIG)
                    t8b = gsb.tile([128, 8], F32, tag="t8b")
                    nc.vector.max(t8b[:, :], l2[:, :])
                    em = gsb.tile([128, E], F32, tag="em")
                    ssum = gsb.tile([128, 1], F32, tag="ssum")
                    nc.vector.scalar_tensor_tensor(
                        out=em[:, :], in0=lg[:, :], scalar=t8b[:, 3:4],
                        in1=e16[:, :], op0=mybir.AluOpType.is_ge,
                        op1=mybir.AluOpType.mult, accum_out=ssum[:, :])
                    rinv = gsb.tile([128, 1], F32, tag="rinv")
                    nc.vector.reciprocal(rinv[:, :], ssum[:, :])
                    g = gsb.tile([128, E], F32, tag="g")
                    nc.vector.tensor_scalar_mul(g[:, :], em[:, :], rinv[:, :])
                    # transposing cast DMA: gates straight to the fp8 DRAM
                    # table (row = expert, col = token); replaces the PE
                    # transpose + copies + flush
                    nc.gpsimd.dma_start(
                        out=bass.AP(tensor=gdram.tensor, offset=128 * i,
                                    ap=[[1, 128], [BC, E]]),
                        in_=g[:, :])

            # ---------------- main pipeline ----------------
            with tc.tile_pool(name="moeps", bufs=DC, space="PSUM") as moeps, \
                 tc.tile_pool(name="hps", bufs=2, space="PSUM") as hps, \
                 tc.tile_pool(name="hsb", bufs=4) as hsbpool, \
                 tc.tile_pool(name="hg8", bufs=2) as hg8pool, \
                 tc.tile_pool(name="opool", bufs=2) as opool:

                hg_tiles = {}
                moe_tiles = {}
                out_ps_box = {}
                g8_tiles = {}

                def load_g8(t, eng):
                    # b2-close rhs: gates as [8, 2, NT] fp8 (e = p + 8*blk)
                    g8 = gsb.tile([8, 2, NT], F8, tag="g8", name="g8")
                    eng.dma_start(
                        out=g8[:, :, :],
                        in_=bass.AP(tensor=gdram.tensor, offset=NT * t,
                                    ap=[[BC, 8], [8 * BC, 2], [1, NT]]))
                    g8_tiles[t] = g8

                def w1_chunk(t, q, m):
                    # 3 W1 DoubleRows -> relu+bias (ACT or DVE) -> Pool gate
                    # multiply into hg[(t,q)][:, m, :] (fp8)
                    ts = slice(NT * t, NT * (t + 1))
                    k = QCHUNK[q][m]
                    hp = hps.tile([128, NT], F32, tag="h", name="h")
                    for c2 in range(3):
                        nc.tensor.matmul(hp[:, :],
                                         w1sb[:, 2 * c2:2 * c2 + 2,
                                              128 * k:128 * (k + 1)],
                                         x8sb[:, 2 * c2:2 * c2 + 2, ts],
                                         start=(c2 == 0), stop=(c2 == 2),
                                         perf_mode=DR)
                    hs_t = hsbpool.tile([128, NT], F32, tag="hs")
                    if m in (1, 4):   # 2 of 6 relus per quad go to DVE
                        nc.vector.scalar_tensor_tensor(
                            out=hs_t[:, :], in0=hp[:, :],
                            scalar=b1sb[:, k:k + 1], in1=zeros[:, :],
                            op0=mybir.AluOpType.add, op1=mybir.AluOpType.max)
                    else:
                        nc.scalar.activation(hs_t[:, :], hp[:, :],
                                             mybir.ActivationFunctionType.Relu,
                                             bias=b1sb[:, k:k + 1])
                    nc.gpsimd.tensor_tensor(
                        out=hg_tiles[(t, q)][:, m, :], in0=hs_t[:, :],
                        in1=gb_tiles[t][:, k, :], op=mybir.AluOpType.mult)

                def w2_slot(t, q, m):
                    # 3 of quad q's 18 W2 DoubleRows (pair-major order)
                    moe = moe_tiles[t]
                    for idx in range(3 * m, 3 * m + 3):
                        j, c = divmod(idx, DC)
                        kp = 4 * q + 2 * j if j < 2 else 16 + 2 * q
                        nc.tensor.matmul(moe[c][:, :],
                                         w2sb[:, kp:kp + 2,
                                              128 * c:128 * (c + 1)],
                                         hg_tiles[(t, q)][:, 2 * j:2 * j + 2, :],
                                         start=(q == 0 and j == 0),
                                         stop=False, perf_mode=DR)

                def stage1(t, q, prev=None, post_m=None):
                    gb = gb_tiles.get(t)
                    hg_tiles[(t, q)] = hg8pool.tile([128, DC, NT], F8,
                                                    tag="hg", name="hg")
                    for m in range(DC):
                        w1_chunk(t, q, m)
                        if prev is not None:
                            w2_slot(t, prev[1], m)
                        if post_m is not None and m in post_m:
                            post_m[m]()

                def finish_chunk(t, c):
                    # z = relu(moe) + x in one DVE op (bf16 out, in place)
                    ts = slice(NT * t, NT * (t + 1))
                    moe = moe_tiles[t]
                    nc.vector.scalar_tensor_tensor(
                        out=xsb[:, c, ts], in0=moe[c][:, :], scalar=0.0,
                        in1=xsb[:, c, ts], op0=mybir.AluOpType.max,
                        op1=mybir.AluOpType.add)

                def head_chunk(t, c):
                    ts = slice(NT * t, NT * (t + 1))
                    if t not in out_ps_box:
                        out_ps_box[t] = moeps.tile([O, NT], F32, tag="moe",
                                                   name="out_ps")
                    nc.tensor.matmul(out_ps_box[t][:, :], wosb[:, c, :],
                                     xsb[:, c, ts],
                                     start=(c == 0), stop=(c == DC - 1))

                def close_tile(t, interleave_next=False):
                    ts = slice(NT * t, NT * (t + 1))
                    moe = moe_tiles[t]
                    for c in range(DC):
                        if interleave_next and c == 0:
                            gb_alloc(t + 1)
                            gb_pure(t + 1, 0, 4, nc.sync)
                            gb_mixed(t + 1, nc.sync)
                            load_g8(t + 1, nc.sync)
                            hg_tiles[(t + 1, 0)] = hg8pool.tile(
                                [128, DC, NT], F8, tag="hg", name="hg")
                        for j in range(3):   # quad 3's W2, c-major
                            kp = 12 + 2 * j if j < 2 else 22
                            nc.tensor.matmul(moe[c][:, :],
                                             w2sb[:, kp:kp + 2,
                                                  128 * c:128 * (c + 1)],
                                             hg_tiles[(t, 3)][:, 2 * j:2 * j + 2, :],
                                             start=False, stop=False,
                                             perf_mode=DR)
                        # b2 bias close: fp8 DoubleRow over the 16 experts
                        nc.tensor.matmul(moe[c][:, :],
                                         b2sb[:, :, 128 * c:128 * (c + 1)],
                                         g8_tiles[t][:, :, :],
                                         start=False, stop=True, perf_mode=DR)
                        finish_chunk(t, c)
                        if c >= 1:
                            head_chunk(t, c - 1)
                        if interleave_next:
                            w1_chunk(t + 1, 0, c)
                    head_chunk(t, DC - 1)
                    osb = opool.tile([O, NT], F32, tag="osb")
                    nc.scalar.activation(osb[:, :], out_ps_box[t][:, :],
                                         mybir.ActivationFunctionType.Identity,
                                         bias=bosb[:, :])
                    nc.sync.dma_start(out=outT[:, ts], in_=osb[:, :])

                # ---- driver ----
                gating_half(0, hps)
                load_w2m(nc.scalar)
                gb_alloc(0)
                gb_pure(0, 0, 1, nc.gpsimd)
                gb_mixed(0, nc.gpsimd)
                load_g8(0, nc.gpsimd)

                moe_tiles[0] = [moeps.tile([128, NT], F32, tag="moe",
                                           name="moe") for _ in range(DC)]
                stage1(0, 0,
                       post_m={5: lambda: gb_pure(0, 1, 2, nc.gpsimd)})
                stage1(0, 1, prev=(0, 0),
                       post_m={1: lambda: load_w2q(1),
                               5: lambda: gb_pure(0, 2, 3, nc.gpsimd)})
                stage1(0, 2, prev=(0, 1),
                       post_m={1: lambda: load_w2q(2),
                               5: lambda: gb_pure(0, 3, 4, nc.gpsimd)})
                # second-half gating issued mid-pipeline (engines in-order;
                # its PE/ACT/DVE slices fit the per-quad slack here)
                gating_half(1, hps)
                stage1(0, 3, prev=(0, 2), post_m={1: lambda: load_w2q(3)})
                close_tile(0, interleave_next=True)

                moe_tiles[1] = [moeps.tile([128, NT], F32, tag="moe",
                                           name="moe") for _ in range(DC)]
                stage1(1, 1, prev=(1, 0))
                stage1(1, 2, prev=(1, 1))
                stage1(1, 3, prev=(1, 2))
                close_tile(1)

            gate_ctxs[0].__exit__(None, None, None)
            ctx_gb.__exit__(None, None, None)

    nc.compile()
    return nc


def _pack_core_inputs(x, Wg, W1, b1, W2, b2, Wo, bo, c4):
    """Per-core input dict for one modality's weights + 1024-token slice."""
    f = np.float32
    tok = slice(BC * c4, BC * (c4 + 1))
    xt = np.ascontiguousarray(np.asarray(x[tok], f).T)
    w1f = np.asarray(W1, f).transpose(1, 0, 2).reshape(D, E * H)[:, HPERM]
    w2f = np.asarray(W2, f).reshape(E * H, D)[HPERM, :]
    b1f = np.asarray(b1, f).reshape(E * H)[HPERM]
    b2f = np.asarray(b2, f)          # [16, D]; row e -> [e % 8, (e//8)*D]
    b28 = np.concatenate([b2f[0:8], b2f[8:16]], axis=1)
    return {
        "xbf": xt.astype(NPBF),
        "x8d": xt.astype(NPF8),
        "w1p": np.ascontiguousarray(w1f.astype(NPF8)),
        "w2p": np.ascontiguousarray(w2f.astype(NPF8)),
        "b1p": np.ascontiguousarray(b1f.reshape(NCH, 128).T),
        "b28": np.ascontiguousarray(b28.astype(NPF8)),
        "wg": np.ascontiguousarray(np.asarray(Wg, f).astype(NPBF)),
        "wo": np.ascontiguousarray(np.asarray(Wo, f).astype(NPBF)),
        "bo": np.ascontiguousarray(np.asarray(bo, f).reshape(O, 1)),
    }


def run_on_hw(inputs, trace=False, **kw):
    if "nc" not in _NC_CACHE:
        _NC_CACHE["nc"] = build_nc()
    nc = _NC_CACHE["nc"]
    in_maps = []
    for core in range(NCORES):
        i, c4 = divmod(core, 4)
        x = inputs["x0"] if i == 0 else inputs["x1"]
        in_maps.append(_pack_core_inputs(
            x, inputs["Wg"][i], inputs["W1"][i], inputs["b1"][i],
            inputs["W2"][i], inputs["b2"][i], inputs["Wo"][i], inputs["bo"][i], c4))
    res = run_bass_kernel_spmd(nc, in_maps, core_ids=list(range(NCORES)),
                               trace=trace, **kw)
    outs = []
    for i in range(2):
        outs.append(np.concatenate(
            [res.results[4 * i + c]["outT"].T for c in range(4)], axis=0))
    return (outs[0], outs[1]), res


def kernel(**inputs):
    (o0, o1), _ = run_on_hw(inputs)
    return (o0, o1)


# revision 19
# speedup vs baseline: 2.5651x; 1.0268x over previous
"""Trainium2 Bass kernel for nn_ClassifierGuided (2-modality top-12-of-16 MoE classifier).

Sharding: pure data-parallel over tokens. 2 modalities x 4096 tokens = 8192
tokens; each of the 8 cores owns 1024 tokens of one modality (cores 0-3 ->
modality 0, cores 4-7 -> modality 1) and that modality's full weights.
Dense-eval MoE (all 16 experts computed, sparse gates applied), so no
all-to-all is needed.

Precision: expert MLP + b2 close in fp8 e4m3 via DoubleRow matmuls (two
128-deep contraction planes per instruction at 0.5 cycles/row = 4x the fp32r
rate). Gating, residual and head run in bf16; top-12 selection flips are rare
near-ties with negligible gate deltas. Measured end-to-end error ~4e-3
against the fp32 reference (tolerance 2e-2).

Layout: 24 h-chunks of 128. Chunks 0-15 are single-expert ("pure": expert e
keeps h[0:128] if e even else h[64:192]); chunks 16-23 are half/half mixed
(expert 2j h[128:192] on partitions 0:64, expert 2j+1 h[0:64] on 64:128).
Gates stream to DRAM as fp8 and come back as a per-chunk broadcast table
[128, 24, NT] in 3 strided DMAs, so the gate multiply is one Pool op per
chunk. Quad q = experts 4q..4q+3 = chunks [4q..4q+4) + [16+2q, 17+2q].

Pipeline: W2 DoubleRows of quad q-1 interleave with W1 chunks of quad q so
the in-order PE never stalls on the 2-bank h-PSUM rotation; relu+bias splits
between ACT (4/quad) and DVE (2/quad); tile 1's first quad runs inside tile
0's close; gating for the second token half is issued mid-pipeline.
"""
import sys

sys.path.insert(0, "/opt/trn_rl_repo")

import numpy as np
import ml_dtypes

import concourse.bass as bass
import concourse.mybir as mybir
import concourse.tile as tile
from concourse import bacc
from concourse.bass_utils import run_bass_kernel_spmd
from concourse.masks import make_identity

# ---- problem sizes (hardcoded per the harness contract) ----
B = 4096           # tokens per modality
D = 768            # model dim
E = 16             # experts
H = 192            # expert hidden
O = 101            # classifier out
KTOP = 12          # top-k experts
NCORES = 8
BC = B // 4        # 1024 tokens per core
DC = D // 128      # 6 d-chunks
NT = 512           # token tile (matmul moving dim / PSUM bank)
NTILES = BC // NT  # 2
NQ = 4             # expert quads
NCH = E * H // 128  # 24 h-chunks
F32 = mybir.dt.float32
BF16 = mybir.dt.bfloat16
F8 = mybir.dt.float8e4
DR = mybir.MatmulPerfMode.DoubleRow
NEG_BIG = -1.0e30
NPF8 = ml_dtypes.float8_e4m3
NPBF = ml_dtypes.bfloat16

_NC_CACHE = {}


def _hperm():
    """Global h-permutation: 16 pure chunks then 8 mixed chunks."""
    idx = []
    for e in range(E):
        lo = 0 if e % 2 == 0 else 64
        idx.extend(e * H + h for h in range(lo, lo + 128))
    for j in range(8):
        idx.extend((2 * j) * H + h for h in range(128, 192))
        idx.extend((2 * j + 1) * H + h for h in range(0, 64))
    return np.array(idx)


HPERM = _hperm()
# quad q covers chunks [4q, 4q+1, 4q+2, 4q+3, 16+2q, 17+2q]
QCHUNK = [[4 * q, 4 * q + 1, 4 * q + 2, 4 * q + 3, 16 + 2 * q, 17 + 2 * q]
          for q in range(NQ)]


def build_nc():
    nc = bacc.Bacc("TRN2", target_bir_lowering=False, debug=False,
                   num_devices=NCORES)

    # ---- DRAM I/O (per-core views; host pre-packs + pre-quantizes) ----
    xbf = nc.dram_tensor("xbf", [D, BC], BF16, kind="ExternalInput").ap()
    x8d = nc.dram_tensor("x8d", [D, BC], F8, kind="ExternalInput").ap()
    w1p = nc.dram_tensor("w1p", [D, E * H], F8, kind="ExternalInput").ap()
    w2p = nc.dram_tensor("w2p", [E * H, D], F8, kind="ExternalInput").ap()
    b1p = nc.dram_tensor("b1p", [128, NCH], F32, kind="ExternalInput").ap()
    b28 = nc.dram_tensor("b28", [8, 2 * D], F8, kind="ExternalInput").ap()
    wg = nc.dram_tensor("wg", [D, E], BF16, kind="ExternalInput").ap()
    wo = nc.dram_tensor("wo", [D, O], BF16, kind="ExternalInput").ap()
    bo = nc.dram_tensor("bo", [O, 1], F32, kind="ExternalInput").ap()
    outT = nc.dram_tensor("outT", [O, BC], F32, kind="ExternalOutput").ap()

    xv = xbf.rearrange("(c p) b -> p c b", p=128)
    x8v = x8d.rearrange("(c p) b -> p c b", p=128)
    w1v = w1p.rearrange("(c p) h -> p c h", p=128)
    w2v = w2p.rearrange("(k p) d -> p k d", p=128)
    wgv = wg.rearrange("(c p) e -> p c e", p=128)
    wov = wo.rearrange("(c p) o -> p c o", p=128)

    with tile.TileContext(nc) as tc:
        with tc.tile_pool(name="const", bufs=1) as cpool:
            xsb = cpool.tile([128, DC, BC], BF16)       # x, later z in place
            x8sb = cpool.tile([128, DC, BC], F8)
            w1sb = cpool.tile([128, DC, E * H], F8)
            w2sb = cpool.tile([128, NCH, D], F8)
            b1sb = cpool.tile([128, NCH], F32)
            b2sb = cpool.tile([8, 2, D], F8)
            wgsb = cpool.tile([128, DC, E], BF16)
            wosb = cpool.tile([128, DC, O], BF16)
            bosb = cpool.tile([O, 1], F32)
            zeros = cpool.tile([128, NT], F32)
            gdram = cpool.tile([E, BC], F8, space="DRAM")

            nc.vector.memset(zeros[:, :], 0.0)

            # ---- load schedule (SP + ACT hwdge queues; gb reads on both
            # SP and Pool). Order is critical: engines are in-order, and a
            # queued DMA blocks later compute on the same engine. ----
            def xq(i, eng):   # quarter of xbf (256 tokens, innermost 512B)
                eng.dma_start(out=xsb[:, :, 256 * i:256 * (i + 1)],
                              in_=xv[:, :, 256 * i:256 * (i + 1)])

            def load_w1q(q):
                HQ = 4 * H
                nc.sync.dma_start(out=w1sb[:, :, HQ * q:HQ * (q + 1)],
                                  in_=w1v[:, :, HQ * q:HQ * (q + 1)])

            def load_w2(k0, k1):   # W2 chunk-rows k0:k1
                nc.sync.dma_start(out=w2sb[:, k0:k1, :], in_=w2v[:, k0:k1, :])

            # SP queue carries every weight in deadline order; the ACT queue
            # stays clear so gating's exp ops run at ~3us.
            nc.sync.dma_start(out=wgsb[:, :, :], in_=wgv)
            xq(1, nc.sync)
            nc.sync.dma_start(out=x8sb[:, :, 0:NT], in_=x8v[:, :, 0:NT])
            load_w1q(0)
            nc.sync.dma_start(out=b1sb[:, :], in_=b1p)
            load_w1q(1)
            load_w2(0, 4)
            load_w2(4, 8)
            load_w1q(2)
            load_w2(16, 20)
            load_w1q(3)
            load_w2(8, 12)
            load_w2(20, 24)
            nc.sync.dma_start(out=x8sb[:, :, NT:], in_=x8v[:, :, NT:])
            load_w2(12, 16)
            nc.sync.dma_start(out=wosb[:, :, :], in_=wov)
            nc.sync.dma_start(out=b2sb[:, :, :],
                              in_=b28.rearrange("p (k d) -> p k d", k=2))
            nc.sync.dma_start(out=bosb[:, :], in_=bo)

            xq(0, nc.scalar)

            # gate-broadcast table reads: fp8 gates round-trip through DRAM,
            # partition-step-0 reads build gball [128, 24, NT]
            gb_tiles = {}
            ctx_gb = tc.tile_pool(name="gball", bufs=2)
            gbpool = ctx_gb.__enter__()

            def gb_alloc(t):
                gb_tiles[t] = gbpool.tile([128, NCH, NT], F8, tag="gb",
                                          name="gball")
                return gb_tiles[t]

            def gb_pure(t, q0, q1, eng):
                # pure chunk cols q0*4 : q1*4 (rows = experts, stride BC)
                gb = gb_tiles[t]
                eng.dma_start(
                    out=gb[:, 4 * q0:4 * q1, :],
                    in_=bass.AP(tensor=gdram.tensor,
                                offset=4 * q0 * BC + NT * t,
                                ap=[[0, 128], [BC, 4 * (q1 - q0)], [1, NT]]))

            def gb_mixed(t, eng):
                # mixed cols 16:24: even expert rows on partitions 0:64,
                # odd expert rows on partitions 64:128
                gb = gb_tiles[t]
                eng.dma_start(
                    out=gb[0:64, 16:24, :],
                    in_=bass.AP(tensor=gdram.tensor, offset=NT * t,
                                ap=[[0, 64], [2 * BC, 8], [1, NT]]))
                eng.dma_start(
                    out=gb[64:128, 16:24, :],
                    in_=bass.AP(tensor=gdram.tensor, offset=BC + NT * t,
                                ap=[[0, 64], [2 * BC, 8], [1, NT]]))

            # ---------------- gating (bf16 logits, exact-enough top-12) ----
            gate_ctxs = [tc.tile_pool(name="gsb", bufs=3)]
            gsb = gate_ctxs[0].__enter__()

            def gating_half(hf, hps):
                for i in range(4 * hf, 4 * hf + 4):
                    ts = slice(128 * i, 128 * (i + 1))
                    lg_ps = hps.tile([128, E], F32, tag="h", name="lg_ps")
                    for c in range(DC):
                        nc.tensor.matmul(lg_ps[:, :], xsb[:, c, ts],
                                         wgsb[:, c, :],
                                         start=(c == 0), stop=(c == DC - 1))
                    lg = gsb.tile([128, E], F32, tag="lg_sb")
                    nc.vector.tensor_copy(lg[:, :], lg_ps[:, :])
                    # exp on ACT runs concurrently with the DVE top-k chain
                    e16 = gsb.tile([128, E], F32, tag="e16")
                    nc.scalar.activation(e16[:, :], lg[:, :],
                                         mybir.ActivationFunctionType.Exp)
                    t8a = gsb.tile([128, 8], F32, tag="t8a")
                    nc.vector.max(t8a[:, :], lg[:, :])
                    l2 = gsb.tile([128, E], F32, tag="l2")
                    nc.vector.match_replace(l2[:, :], t8a[:, :], lg[:, :],
                                            NEG_BIG)
                    t8b = gsb.tile([128, 8], F32, tag="t8b")
                    nc.vector.max(t8b[:, :], l2[:, :])
                    em = gsb.tile([128, E], F32, tag="em")
                    ssum = gsb.tile([128, 1], F32, tag="ssum")
                    nc.vector.scalar_tensor_tensor(
                        out=em[:, :], in0=lg[:, :], scalar=t8b[:, 3:4],
                        in1=e16[:, :], op0=mybir.AluOpType.is_ge,
                        op1=mybir.AluOpType.mult, accum_out=ssum[:, :])
                    rinv = gsb.tile([128, 1], F32, tag="rinv")
                    nc.vector.reciprocal(rinv[:, :], ssum[:, :])
                    g = gsb.tile([128, E], F32, tag="g")
                    nc.vector.tensor_scalar_mul(g[:, :], em[:, :], rinv[:, :])
                    # transposing cast DMA: gates straight to the fp8 DRAM
                    # table (row = expert, col = token); replaces the PE
                    # transpose + copies + flush
                    nc.gpsimd.dma_start(
                        out=bass.AP(tensor=gdram.tensor, offset=128 * i,
                                    ap=[[1, 128], [BC, E]]),
                        in_=g[:, :])

            # ---------------- main pipeline ----------------
            with tc.tile_pool(name="moeps", bufs=DC, space="PSUM") as moeps, \
                 tc.tile_pool(name="hps", bufs=2, space="PSUM") as hps, \
                 tc.tile_pool(name="hsb", bufs=4) as hsbpool, \
                 tc.tile_pool(name="hg8", bufs=2) as hg8pool, \
                 tc.tile_pool(name="opool", bufs=2) as opool:

                hg_tiles = {}
                moe_tiles = {}
                out_ps_box = {}
                g8_tiles = {}

                def load_g8(t, eng):
                    # b2-close rhs: gates as [8, 2, NT] fp8 (e = p + 8*blk)
                    g8 = gsb.tile([8, 2, NT], F8, tag="g8", name="g8")
                    eng.dma_start(
                        out=g8[:, :, :],
                        in_=bass.AP(tensor=gdram.tensor, offset=NT * t,
                                    ap=[[BC, 8], [8 * BC, 2], [1, NT]]))
                    g8_tiles[t] = g8

                def w1_chunk(t, q, m):
                    # 3 W1 DoubleRows -> relu+bias (ACT or DVE) -> Pool gate
                    # multiply into hg[(t,q)][:, m, :] (fp8)
                    ts = slice(NT * t, NT * (t + 1))
                    k = QCHUNK[q][m]
                    hp = hps.tile([128, NT], F32, tag="h", name="h")
                    for c2 in range(3):
                        nc.tensor.matmul(hp[:, :],
                                         w1sb[:, 2 * c2:2 * c2 + 2,
                                              128 * k:128 * (k + 1)],
                                         x8sb[:, 2 * c2:2 * c2 + 2, ts],
                                         start=(c2 == 0), stop=(c2 == 2),
                                         perf_mode=DR)
                    hs_t = hsbpool.tile([128, NT], F32, tag="hs")
                    if m in (1, 4):   # 2 of 6 relus per quad go to DVE
                        nc.vector.scalar_tensor_tensor(
                            out=hs_t[:, :], in0=hp[:, :],
                            scalar=b1sb[:, k:k + 1], in1=zeros[:, :],
                            op0=mybir.AluOpType.add, op1=mybir.AluOpType.max)
                    else:
                        nc.scalar.activation(hs_t[:, :], hp[:, :],
                                             mybir.ActivationFunctionType.Relu,
                                             bias=b1sb[:, k:k + 1])
                    nc.gpsimd.tensor_tensor(
                        out=hg_tiles[(t, q)][:, m, :], in0=hs_t[:, :],
                        in1=gb_tiles[t][:, k, :], op=mybir.AluOpType.mult)

                def w2_slot(t, q, m):
                    # 3 of quad q's 18 W2 DoubleRows (pair-major order)
                    moe = moe_tiles[t]
                    for idx in range(3 * m, 3 * m + 3):
                        j, c = divmod(idx, DC)
                        kp = 4 * q + 2 * j if j < 2 else 16 + 2 * q
                        nc.tensor.matmul(moe[c][:, :],
                                         w2sb[:, kp:kp + 2,
                                              128 * c:128 * (c + 1)],
                                         hg_tiles[(t, q)][:, 2 * j:2 * j + 2, :],
                                         start=(q == 0 and j == 0),
                                         stop=False, perf_mode=DR)

                def stage1(t, q, prev=None, post_m=None):
                    gb = gb_tiles.get(t)
                    hg_tiles[(t, q)] = hg8pool.tile([128, DC, NT], F8,
                                                    tag="hg", name="hg")
                    for m in range(DC):
                        w1_chunk(t, q, m)
                        if prev is not None:
                            w2_slot(t, prev[1], m)
                        if post_m is not None and m in post_m:
                            post_m[m]()

                def finish_chunk(t, c):
                    # z = relu(moe) + x in one DVE op (bf16 out, in place)
                    ts = slice(NT * t, NT * (t + 1))
                    moe = moe_tiles[t]
                    nc.vector.scalar_tensor_tensor(
                        out=xsb[:, c, ts], in0=moe[c][:, :], scalar=0.0,
                        in1=xsb[:, c, ts], op0=mybir.AluOpType.max,
                        op1=mybir.AluOpType.add)

                def head_chunk(t, c):
                    ts = slice(NT * t, NT * (t + 1))
                    if t not in out_ps_box:
                        out_ps_box[t] = moeps.tile([O, NT], F32, tag="moe",
                                                   name="out_ps")
                    nc.tensor.matmul(out_ps_box[t][:, :], wosb[:, c, :],
                                     xsb[:, c, ts],
                                     start=(c == 0), stop=(c == DC - 1))

                def close_tile(t, interleave_next=False):
                    ts = slice(NT * t, NT * (t + 1))
                    moe = moe_tiles[t]
                    # part A: final quad's j=0,1 DoubleRows (only need the
                    # quad's first 4 hg chunks) + next tile's first W1 quad
                    for c in range(DC):
                        if interleave_next and c == 0:
                            gb_alloc(t + 1)
                            gb_pure(t + 1, 0, 4, nc.sync)
                            gb_mixed(t + 1, nc.sync)
                            load_g8(t + 1, nc.sync)
                            hg_tiles[(t + 1, 0)] = hg8pool.tile(
                                [128, DC, NT], F8, tag="hg", name="hg")
                        for j in range(2):
                            nc.tensor.matmul(moe[c][:, :],
                                             w2sb[:, 12 + 2 * j:14 + 2 * j,
                                                  128 * c:128 * (c + 1)],
                                             hg_tiles[(t, 3)][:, 2 * j:2 * j + 2, :],
                                             start=False, stop=False,
                                             perf_mode=DR)
                        if interleave_next:
                            w1_chunk(t + 1, 0, c)
                    # part B: per-chunk mixed-pair close, b2 bias, residual
                    # drain, trailing head
                    for c in range(DC):
                        nc.tensor.matmul(moe[c][:, :],
                                         w2sb[:, 22:24, 128 * c:128 * (c + 1)],
                                         hg_tiles[(t, 3)][:, 4:6, :],
                                         start=False, stop=False, perf_mode=DR)
                        nc.tensor.matmul(moe[c][:, :],
                                         b2sb[:, :, 128 * c:128 * (c + 1)],
                                         g8_tiles[t][:, :, :],
                                         start=False, stop=True, perf_mode=DR)
                        finish_chunk(t, c)
                        if c >= 1:
                            head_chunk(t, c - 1)
                    head_chunk(t, DC - 1)
                    osb = opool.tile([O, NT], F32, tag="osb")
                    nc.scalar.activation(osb[:, :], out_ps_box[t][:, :],
                                         mybir.ActivationFunctionType.Identity,
                                         bias=bosb[:, :])
                    nc.sync.dma_start(out=outT[:, ts], in_=osb[:, :])

                # ---- driver ----
                gating_half(0, hps)
                gb_alloc(0)
                gb_pure(0, 0, 1, nc.gpsimd)
                gb_mixed(0, nc.gpsimd)
                load_g8(0, nc.gpsimd)

                moe_tiles[0] = [moeps.tile([128, NT], F32, tag="moe",
                                           name="moe") for _ in range(DC)]
                stage1(0, 0,
                       post_m={1: lambda: xq(2, nc.scalar),
                               5: lambda: gb_pure(0, 1, 2, nc.gpsimd)})
                stage1(0, 1, prev=(0, 0),
                       post_m={1: lambda: xq(3, nc.scalar),
                               5: lambda: gb_pure(0, 2, 3, nc.gpsimd)})
                stage1(0, 2, prev=(0, 1),
                       post_m={5: lambda: gb_pure(0, 3, 4, nc.gpsimd)})
                # second-half gating issued mid-pipeline (engines in-order;
                # its PE/ACT/DVE slices fit the per-quad slack here)
                gating_half(1, hps)
                stage1(0, 3, prev=(0, 2))
                close_tile(0, interleave_next=True)

                moe_tiles[1] = [moeps.tile([128, NT], F32, tag="moe",
                                           name="moe") for _ in range(DC)]
                stage1(1, 1, prev=(1, 0))
                stage1(1, 2, prev=(1, 1))
                stage1(1, 3, prev=(1, 2))
                close_tile(1)

            gate_ctxs[0].__exit__(None, None, None)
            ctx_gb.__exit__(None, None, None)

    nc.compile()
    return nc


def _pack_core_inputs(x, Wg, W1, b1, W2, b2, Wo, bo, c4):
    """Per-core input dict for one modality's weights + 1024-token slice."""
    f = np.float32
    tok = slice(BC * c4, BC * (c4 + 1))
    xt = np.ascontiguousarray(np.asarray(x[tok], f).T)
    w1f = np.asarray(W1, f).transpose(1, 0, 2).reshape(D, E * H)[:, HPERM]
    w2f = np.asarray(W2, f).reshape(E * H, D)[HPERM, :]
    b1f = np.asarray(b1, f).reshape(E * H)[HPERM]
    b2f = np.asarray(b2, f)          # [16, D]; row e -> [e % 8, (e//8)*D]
    b28 = np.concatenate([b2f[0:8], b2f[8:16]], axis=1)
    return {
        "xbf": xt.astype(NPBF),
        "x8d": xt.astype(NPF8),
        "w1p": np.ascontiguousarray(w1f.astype(NPF8)),
        "w2p": np.ascontiguousarray(w2f.astype(NPF8)),
        "b1p": np.ascontiguousarray(b1f.reshape(NCH, 128).T),
        "b28": np.ascontiguousarray(b28.astype(NPF8)),
        "wg": np.ascontiguousarray(np.asarray(Wg, f).astype(NPBF)),
        "wo": np.ascontiguousarray(np.asarray(Wo, f).astype(NPBF)),
        "bo": np.ascontiguousarray(np.asarray(bo, f).reshape(O, 1)),
    }


def run_on_hw(inputs, trace=False, **kw):
    if "nc" not in _NC_CACHE:
        _NC_CACHE["nc"] = build_nc()
    nc = _NC_CACHE["nc"]
    in_maps = []
    for core in range(NCORES):
        i, c4 = divmod(core, 4)
        x = inputs["x0"] if i == 0 else inputs["x1"]
        in_maps.append(_pack_core_inputs(
            x, inputs["Wg"][i], inputs["W1"][i], inputs["b1"][i],
            inputs["W2"][i], inputs["b2"][i], inputs["Wo"][i], inputs["bo"][i], c4))
    res = run_bass_kernel_spmd(nc, in_maps, core_ids=list(range(NCORES)),
                               trace=trace, **kw)
    outs = []
    for i in range(2):
        outs.append(np.concatenate(
            [res.results[4 * i + c]["outT"].T for c in range(4)], axis=0))
    return (outs[0], outs[1]), res


def kernel(**inputs):
    (o0, o1), _ = run_on_hw(inputs)
    return (o0, o1)
